# revision 38
# baseline (speedup 1.0000x reference)
"""KANFIS forward on 8 NeuronCores, data-parallel over the batch.

Per core (batch shard 16384 rows, processed as 16 chunk-pairs of 1024 so
every elementwise op runs at free-dim 1024 and amortizes engine overhead):

phase1  x (contiguous DMA; batch order permuted to col=j*128+p, undone at
        the output DMA) -> 8 PE transposes give x^T in PSUM partitions
        0:64; ACT Square writes the k=0 gaussian arg into 64:128; ONE
        packed ACT Exp produces [P ; rbf_0] (P = exp(dc*x/sigma^2), with
        per-partition scale switching function halves). The remaining 7
        RBF planes come from the squared-step DVE chain (bf16, 2x mode):
        X1=[t0;t1] via one mult + copies, PP2=[P^2;P^2], then
        X_{j+1} = X_j (.) PP2 yields plane pairs {t2,t3},{t4,t5},{t6,t7},
        where t_m = rbf_m / gamma_m and gamma folds into the baked bf16
        matmul weights. Group projection + k-sum = 4 paired bf16 matmuls
        + one float32r matmul for the linear term. BN1 partials: S1 via
        ACT accum on the PSUM->SBUF evict (applies proj bias, stores
        projT bf16), Q1 via batched ACT Square accum; per-pair partials
        land in scol columns, one DVE reduce at the end.
AR1     AllReduce [128,2]; a1 = g1*exp(-0.5*ln(var+eps)) (Ln+Exp share the
        natural_log_exp ACT table set - no table switch), d1 likewise.
phase2  BN1 affine folded into the fp weights: fpw_eff = a1 (.) fp_W^T
        (DVE), biasEff = fp_b + fp_W @ d1 (PE). Two bf16 matmuls per pair
        produce y^T [20,1024] feature-major; ACT evict applies biasEff and
        accumulates S2; DVE square-accum gives Q2.
AR2     AllReduce [20,2] -> a2,d2.
phase3  3a: z^T = Gelu(a2*y^T+d2) per-partition feature-major (no
        transposes), z^2 on DVE -> zzT [52,*] f32r (z at partitions 0:20,
        z^2 at 32:52 - DVE partition bases must be 32-aligned).
        3b: u = afz @ zz (float32r matmuls, 1 cyc/row); memberships: three
        ACT Exp -> bf16 tiles; the e2l plane instead uses a DVE int16
        bit-trick exp (bits16 = A16*clamped_exponent + B16 -> bitcast
        bf16, ~3% rel err, fine for the 2e-2 budget since the fuzzy head
        averages 400 such terms); head-sum via 4 bf16 matmuls into a
        [1,512] PSUM row per chunk, DVE adds head_b while un-permuting
        the batch order, one contiguous 4KB DMA per pair.

All parameters are baked into the NEFF as inline tensors; only x is a
runtime input. BN statistics are computed on device with two small
AllReduces. If the RBF grid is not uniformly spaced with per-group
constant sigma, a direct (non-recurrence) fallback path is built instead.

TimelineSim (cost-model) per-core estimate: ~193 us vs ~646 us for the
v0 baseline; hardware-verified rel err vs the fp64 reference: 3.8e-3.
"""
import numpy as np
import ml_dtypes
from contextlib import ExitStack

import concourse.bass as bass
import concourse.tile as tile
from concourse import mybir
from concourse.vector_clock import ScopedClock
from concourse.bass_utils import run_bass_kernel_spmd

F32 = mybir.dt.float32
F32R = mybir.dt.float32r
BF16 = mybir.dt.bfloat16
AF = mybir.ActivationFunctionType
ALU = mybir.AluOpType
BF = ml_dtypes.bfloat16

NCORES = 8
B = 131072
BS = B // NCORES          # 16384 rows per core
G, GS, K, O = 8, 8, 8, 16
TOT, R, FIN = 128, 10, 20
EPS = 1e-5
FC = 512                  # chunk free size
NCH = BS // FC            # 32 chunks


class SplitDrainTileContext(tile.TileContext):
    """walrus on this stack rejects >1 sync wait per instruction; split the
    kernel-tail drain's waits into single-wait nops."""

    def _drain_and_barrier(self, tick_clock, wait_clock):
        nc = self.nc
        nop = nc.sync.nop(nofuse=True)
        wait_clock.add_sem_waits(nop.ins, ScopedClock({None: tick_clock.global_clock}))
        si = nop.ins.sync_info
        waits = list(si.on_wait) if si and si.on_wait else []
        if len(waits) > 1:
            nop.ins.sync_info = mybir.SyncInfo(on_wait=waits[:1], on_update=si.on_update)
            for w in waits[1:]:
                n2 = nc.sync.nop(nofuse=True)
                n2.ins.sync_info = mybir.SyncInfo(on_wait=[w], on_update=[])
        nc.sync.drain()
        nc.all_engine_barrier()
        assert self.sems is not None
        popped = nc._tile_sem_poison_stack.pop()
        assert popped is self._sem_poison
        nc.clear_and_free_semaphores(list(self.sems.allocated().values()))
        nc.all_engine_barrier()


def _build(p):
    nc = bass.Bass(num_devices=NCORES)
    x = nc.dram_tensor("x", [BS, 64], F32, kind="ExternalInput")
    out = nc.dram_tensor("out", [BS, 1], F32, kind="ExternalOutput")
    ar1_in = nc.dram_tensor("ar1_in", [128, 2], F32)
    ar1_out = nc.dram_tensor("ar1_out", [128, 2], F32)
    ar2_in = nc.dram_tensor("ar2_in", [20, 2], F32)
    ar2_out = nc.dram_tensor("ar2_out", [20, 2], F32)

    # ---- baked constants (numpy) ----
    sig = np.exp(np.asarray(p["rbf_log_widths"], np.float64)) + 1e-6   # [G,K]
    cen = np.asarray(p["rbf_centres"], np.float64)                     # [G,K]
    inv = 1.0 / sig
    # recurrence structure: uniform centre spacing + k-constant sigma per group
    dcen = np.diff(cen, axis=1)
    rec_ok = (K >= 3 and np.allclose(dcen, dcen[:, :1], rtol=1e-5, atol=1e-7)
              and np.allclose(sig, sig[:, :1], rtol=1e-5, atol=1e-9))
    # seed Square scale/bias per partition.
    # Recurrence mode: one [128] vector — partitions 64:128 hold k=0 params
    # (Square input is x^T there; the squared result lands in 0:64).
    # The packed Exp then uses scale -0.5 on 0:64 (gaussian) and the P-scale
    # dc/sigma^2 on 64:128 (reads raw x^T).
    sqs = np.zeros((4, 128), np.float32)
    sqb = np.zeros((4, 128), np.float32)
    for j in range(K // 2):
        for half, k in ((0, 2 * j), (1, 2 * j + 1)):
            s = np.repeat(inv[:, k], GS)                # [64]
            b = np.repeat(-cen[:, k] * inv[:, k], GS)
            sqs[j, half * 64:(half + 1) * 64] = s
            sqb[j, half * 64:(half + 1) * 64] = b
    sqs_rec = np.zeros(128, np.float32)
    sqb_rec = np.zeros(128, np.float32)
    sqs_rec[:64] = np.repeat(inv[:, 0], GS)
    sqb_rec[:64] = np.repeat(-cen[:, 0] * inv[:, 0], GS)
    dc_g = dcen[:, 0] if rec_ok else np.zeros(G)
    exps_rec = np.zeros(128, np.float32)
    exps_rec[:64] = np.repeat(dc_g / sig[:, 0] ** 2, GS)
    exps_rec[64:] = -0.5
    # chain-plane weights: plane m is t_m = rbf_0 * P^m = rbf_m / gamma_m;
    # gamma_m = exp(-(c_m^2 - c_0^2) / (2 sigma^2)) folds into the lhsT.
    pw = np.asarray(p["proj_W"], np.float64)            # [G,O,GS]
    w = np.asarray(p["rbf_weights"], np.float64)        # [G,K]
    if rec_ok:
        gam = np.exp(-(cen ** 2 - cen[:, :1] ** 2) / (2 * sig[:, :1] ** 2))  # [G,K]
        weff = w * gam
    else:
        weff = w
    # lhsT blocks: 4 pairs {2j,2j+1} as [128,128]. In rec mode the planes are
    # t_m = rbf_m / gamma_m and gamma folds into weff.
    lhp = np.zeros((K // 2, 128, 128), np.float64)
    for j in range(K // 2):
        for half, k in ((0, 2 * j), (1, 2 * j + 1)):
            for g in range(G):
                lhp[j, half * 64 + g * GS:half * 64 + (g + 1) * GS,
                    g * O:(g + 1) * O] = pw[g].T * weff[g, k]
    linT = np.zeros((64, 128), np.float32)
    for g in range(G):
        linT[g * GS:(g + 1) * GS, g * O:(g + 1) * O] = (
            pw[g].T * np.asarray(p["rbf_linear_w"], np.float64)[g])
    pbv = np.asarray(p["proj_b"], np.float32).reshape(128, 1)
    # fuzzy layer
    su = np.exp(np.asarray(p["fz_log_su"], np.float64)) + 1e-6          # [R,FIN]
    sl = np.minimum(np.exp(np.asarray(p["fz_log_sl"], np.float64)) + 1e-6, su * 0.9)
    cz = np.asarray(p["fz_centres"], np.float64)
    # z rows at partitions 0:20, z^2 rows at 32:52 (DVE partition bases must
    # be 32-aligned, so z^2 is written at base 32; rows 20:32 stay zero)
    afz = np.zeros((52, 200), np.float32)
    for r in range(R):
        for f in range(FIN):
            m = r * FIN + f
            afz[f, m] = -2.0 * cz[r, f] / su[r, f] ** 2
            afz[32 + f, m] = 1.0 / su[r, f] ** 2
    ubias = (-0.5 * cz ** 2 / su ** 2).reshape(200, 1).astype(np.float32)
    lbias = (-0.5 * cz ** 2 / sl ** 2).reshape(200, 1).astype(np.float32)
    lscale = (-0.5 * (su / sl) ** 2).reshape(200, 1).astype(np.float32)
    wh = np.repeat(np.asarray(p["head_W"], np.float64).reshape(R, 1) * 0.5 / FIN,
                   FIN, 0).astype(np.float32)                           # [200,1]
    # bf16 bit-trick exp for the e2l plane: bits16 = A16*expnt + B16, expnt
    # clamped at -80 (below which the int16 pattern would go negative)
    A16 = 128.0 / np.log(2.0)
    B16 = 16256.0 - 486411.0 / 65536.0
    ls2 = lscale[128:, 0].astype(np.float64)
    lb2 = lbias[128:, 0].astype(np.float64)
    uc2 = ((-80.0 - lb2) / ls2).astype(np.float32).reshape(72, 1)
    als2 = (A16 * ls2).astype(np.float32).reshape(72, 1)
    ablb2 = (A16 * lb2 + B16).astype(np.float32).reshape(72, 1)
    head_b = float(np.asarray(p["head_b"]).reshape(-1)[0])

    def it(name, arr, dt=None):
        a = np.ascontiguousarray(arr)
        if dt == "bf16":
            a = a.astype(BF)
        else:
            a = a.astype(np.float32)
        return nc.inline_tensor(a, name=name)

    c_id = it("c_id", np.eye(128))
    c_sqs = it("c_sqs", sqs.T)           # [128,4]
    c_sqb = it("c_sqb", sqb.T)
    c_sqsr = it("c_sqsr", sqs_rec.reshape(128, 1))
    c_sqbr = it("c_sqbr", sqb_rec.reshape(128, 1))
    c_exsr = it("c_exsr", exps_rec.reshape(128, 1))
    c_lh = it("c_lh", lhp.transpose(1, 0, 2).reshape(128, 4 * 128), "bf16")
    c_lin = it("c_lin", linT)            # [64,128] f32
    c_pb = it("c_pb", pbv)
    c_g1 = it("c_g1", np.asarray(p["bn1_gamma"]).reshape(128, 1))
    c_b1 = it("c_b1", np.asarray(p["bn1_beta"]).reshape(128, 1))
    c_g2 = it("c_g2", np.asarray(p["bn2_gamma"]).reshape(20, 1))
    c_b2 = it("c_b2", np.asarray(p["bn2_beta"]).reshape(20, 1))
    c_fpw = it("c_fpw", np.asarray(p["fp_W"]).T)        # [128,20]
    c_fpb = it("c_fpb", np.asarray(p["fp_b"]).reshape(20, 1))
    c_afz1 = it("c_afz1", afz[:, :128])
    c_afz2 = it("c_afz2", afz[:, 128:])
    c_ub1 = it("c_ub1", ubias[:128]); c_ub2 = it("c_ub2", ubias[128:])
    c_lb1 = it("c_lb1", lbias[:128]); c_lb2 = it("c_lb2", lbias[128:])
    c_ls1 = it("c_ls1", lscale[:128]); c_ls2 = it("c_ls2", lscale[128:])
    c_wh1 = it("c_wh1", wh[:128], "bf16"); c_wh2 = it("c_wh2", wh[128:], "bf16")
    c_uc2 = it("c_uc2", uc2); c_als2 = it("c_als2", als2)
    c_ablb2 = it("c_ablb2", ablb2)

    octx = ExitStack()

    def sb(n, s, dt=F32):
        return octx.enter_context(nc.sbuf_tensor(n, s, dt))

    projT = sb("projT", [128, BS], BF16)     # 4MB persistent
    yT = sb("yT", [20, BS], BF16)
    zzT = sb("zzT", [52, BS], F32R)
    s1a = sb("s1a", [128, 2]); s2a = sb("s2a", [20, 2])
    scol1 = sb("scol1", [128, NCH // 2])     # per-pair S1 accums
    scol2 = sb("scol2", [128, NCH // 4])     # per-group Q1 accums
    scol3 = sb("scol3", [20, NCH // 2]); scol4 = sb("scol4", [20, NCH // 2])
    a1v = sb("a1v", [128, 1]); d1v = sb("d1v", [128, 1])
    a2v = sb("a2v", [20, 1]); d2v = sb("d2v", [20, 1])
    fpw_eff = sb("fpw_eff", [128, 20], BF16)
    biasEff = sb("biasEff", [20, 1])
    # const sbuf copies
    k_id = sb("k_id", [128, 128])
    k_sqs = sb("k_sqs", [128, 4]); k_sqb = sb("k_sqb", [128, 4])
    k_sqsr = sb("k_sqsr", [128, 1]); k_sqbr = sb("k_sqbr", [128, 1])
    k_exsr = sb("k_exsr", [128, 1])
    k_lh = sb("k_lh", [128, 4 * 128], BF16)
    k_lin = sb("k_lin", [64, 128], F32R); k_pb = sb("k_pb", [128, 1])
    k_g1 = sb("k_g1", [128, 1]); k_b1 = sb("k_b1", [128, 1])
    k_g2 = sb("k_g2", [20, 1]); k_b2 = sb("k_b2", [20, 1])
    k_fpw = sb("k_fpw", [128, 20]); k_fpb = sb("k_fpb", [20, 1])
    k_afz1 = sb("k_afz1", [52, 128], F32R); k_afz2 = sb("k_afz2", [52, 72], F32R)
    k_ub1 = sb("k_ub1", [128, 1]); k_ub2 = sb("k_ub2", [72, 1])
    k_lb1 = sb("k_lb1", [128, 1]); k_lb2 = sb("k_lb2", [72, 1])
    k_ls1 = sb("k_ls1", [128, 1]); k_ls2 = sb("k_ls2", [72, 1])
    k_wh1 = sb("k_wh1", [128, 1], BF16); k_wh2 = sb("k_wh2", [72, 1], BF16)
    k_uc2 = sb("k_uc2", [72, 1]); k_als2 = sb("k_als2", [72, 1])
    k_ablb2 = sb("k_ablb2", [72, 1])
    k_e1 = sb("k_e1", [128, 1]); k_e2 = sb("k_e2", [20, 1])
    k_hb4 = sb("k_hb4", [4, 1])

    # ================= phase 1 =================
    with ExitStack() as ctx:
        tc = ctx.enter_context(SplitDrainTileContext(nc))
        # consts via SWDGE in use order so they never block the x loads
        for dst, src in [(k_id, c_id), (k_sqsr, c_sqsr), (k_sqbr, c_sqbr),
                         (k_exsr, c_exsr), (k_lh, c_lh), (k_lin, c_lin),
                         (k_pb, c_pb), (k_g1, c_g1),
                         (k_b1, c_b1), (k_g2, c_g2), (k_b2, c_b2),
                         (k_fpw, c_fpw), (k_fpb, c_fpb), (k_afz1, c_afz1),
                         (k_afz2, c_afz2), (k_ub1, c_ub1), (k_ub2, c_ub2),
                         (k_lb1, c_lb1), (k_lb2, c_lb2), (k_ls1, c_ls1),
                         (k_ls2, c_ls2), (k_wh1, c_wh1), (k_wh2, c_wh2),
                         (k_uc2, c_uc2), (k_als2, c_als2),
                         (k_ablb2, c_ablb2), (k_sqs, c_sqs), (k_sqb, c_sqb)]:
            cast = dst.dtype == F32R
            nc.gpsimd.dma_start(out=dst[:],
                                in_=src[:, :].bitcast(F32R) if cast else src[:, :])
        nc.vector.memset(k_e1[:], EPS)
        nc.vector.memset(k_e2[:], EPS)
        nc.vector.memset(k_hb4[:], head_b)
        pool = ctx.enter_context(tc.tile_pool(name="p1", bufs=3))
        psum = ctx.enter_context(tc.tile_pool(name="ps1", bufs=2, space="PSUM"))
        # chunk pairs: FD=1024 per op; within a pair, column j*128+p holds
        # batch row d*1024 + 8p + j (undone at the output DMA)
        FC2 = 2 * FC
        xv = x.rearrange("(d p s) f -> d p (s f)", p=128, s=8)
        for d in range(NCH // 2):
            xt = pool.tile([128, 512], F32, tag="xt")
            nc.sync.dma_start(out=xt[:], in_=xv[d])
            uix = psum.tile([128, FC2], F32, tag="uix")
            # x^T at partitions 0:64 (transpose may only write partition 0);
            # the squared gaussian arg goes to 64:128
            for j in range(8):
                nc.tensor.transpose(uix[0:64, j * 128:(j + 1) * 128],
                                    xt[:, j * 64:(j + 1) * 64], k_id[:])
            xts = pool.tile([64, FC2], F32R, tag="xts")
            if d % 2 == 0:
                nc.scalar.copy(xts[:], uix[0:64, :])
            else:
                nc.vector.tensor_copy(xts[:], uix[0:64, :])
            pp = psum.tile([128, FC2], F32, tag="pp")
            if rec_ok:
                nc.scalar.activation(uix[64:128, :], uix[0:64, :], AF.Square,
                                     bias=k_sqbr[0:64, 0:1],
                                     scale=k_sqsr[0:64, 0:1])
                # packed exp: [P ; rbf_0]
                e0 = pool.tile([128, FC2], BF16, tag="e0")
                nc.scalar.activation(e0[:], uix[:], AF.Exp, bias=0.0,
                                     scale=k_exsr[:])
                # P duplicated to partition base 64 (DVE needs equal input
                # bases when both tensors are in SBUF)
                ptU = pool.tile([128, FC2], BF16, tag="ptU")
                nc.vector.tensor_copy(ptU[64:128, :], e0[0:64, :])
                # squared-step chain: X1=[t0;t1], PP2=[P^2;P^2],
                # X_{j+1} = X_j * PP2 gives [t2;t3],[t4;t5],[t6;t7]
                # (gamma folded into lhsT)
                eA = pool.tile([128, FC2], BF16, tag="eA")
                eB = pool.tile([128, FC2], BF16, tag="eB")
                eC = pool.tile([128, FC2], BF16, tag="eC")
                eD = pool.tile([128, FC2], BF16, tag="eD")
                pp2 = pool.tile([128, FC2], BF16, tag="pp2")
                nc.vector.tensor_copy(eA[0:64, :], e0[64:128, :])
                nc.vector.tensor_tensor(eA[64:128, :], e0[64:128, :],
                                        ptU[64:128, :], ALU.mult)
                nc.vector.tensor_tensor(pp2[0:64, :], e0[0:64, :], e0[0:64, :],
                                        ALU.mult)
                nc.vector.tensor_copy(pp2[64:128, :], pp2[0:64, :])
                nc.vector.tensor_tensor(eB[:], eA[:], pp2[:], ALU.mult)
                nc.vector.tensor_tensor(eC[:], eB[:], pp2[:], ALU.mult)
                nc.vector.tensor_tensor(eD[:], eC[:], pp2[:], ALU.mult)
                for h in range(2):
                    hs = slice(h * FC, (h + 1) * FC)
                    for j, et in enumerate([eA, eB, eC, eD]):
                        nc.tensor.matmul(pp[:, hs],
                                         k_lh[:, j * 128:(j + 1) * 128],
                                         et[:, hs], start=(j == 0), stop=False)
                    nc.tensor.matmul(pp[:, hs], k_lin[:], xts[:, hs],
                                     start=False, stop=True)
            else:
                for j in range(8):
                    nc.tensor.matmul(uix[64:128, j * 128:(j + 1) * 128],
                                     xt[:, j * 64:(j + 1) * 64], k_id[:],
                                     start=True, stop=True)
                for h in range(2):
                    hs = slice(h * FC, (h + 1) * FC)
                    for j in range(K // 2):
                        uj2 = psum.tile([128, FC], F32, tag="uj", name="uj2")
                        nc.scalar.activation(uj2[:], uix[:, hs], AF.Square,
                                             bias=k_sqb[:, j:j + 1],
                                             scale=k_sqs[:, j:j + 1])
                        ej2 = pool.tile([128, FC], BF16, tag="ej2", name="ej2")
                        nc.scalar.activation(ej2[:], uj2[:], AF.Exp,
                                             bias=0.0, scale=-0.5)
                        nc.tensor.matmul(pp[:, hs],
                                         k_lh[:, j * 128:(j + 1) * 128],
                                         ej2[:], start=(j == 0), stop=False)
                    nc.tensor.matmul(pp[:, hs], k_lin[:], xts[:, hs],
                                     start=False, stop=True)
            nc.scalar.activation(projT[:, d * FC2:(d + 1) * FC2], pp[:],
                                 AF.Identity, bias=k_pb[:], scale=1.0,
                                 accum_out=scol1[:, d:d + 1])
            if d % 2 == 1:
                qscr = pool.tile([128, 4 * FC], BF16, tag="qscr")
                g = d // 2
                nc.scalar.activation(qscr[:], projT[:, g * 4 * FC:(g + 1) * 4 * FC],
                                     AF.Square,
                                     accum_out=scol2[:, g:g + 1])
        nc.vector.reduce_sum(s1a[:, 0:1], scol1[:], axis=mybir.AxisListType.X)
        nc.vector.reduce_sum(s1a[:, 1:2], scol2[:], axis=mybir.AxisListType.X)
        nc.sync.dma_start(out=ar1_in[:, :], in_=s1a[:])

    with nc.semaphore("cc1") as cs:
        nc.gpsimd.collective_compute(
            "AllReduce", ALU.add, replica_groups=[list(range(NCORES))],
            ins=[ar1_in[:, :].opt()], outs=[ar1_out[:, :].opt()]).then_inc(cs, 1)
        nc.gpsimd.wait_ge(cs, 1)
        nc.all_engine_barrier()

    # ================= phase 2 =================
    with ExitStack() as ctx:
        tc = ctx.enter_context(SplitDrainTileContext(nc))
        pool = ctx.enter_context(tc.tile_pool(name="p2", bufs=3))
        psum = ctx.enter_context(tc.tile_pool(name="ps2", bufs=2, space="PSUM"))
        sq1 = pool.tile([128, 2], F32)
        nc.sync.dma_start(out=sq1[:], in_=ar1_out[:, :])
        mu = pool.tile([128, 1], F32)
        nc.scalar.mul(mu[:], sq1[:, 0:1], 1.0 / B)
        mus = pool.tile([128, 1], F32)
        nc.vector.tensor_mul(mus[:], mu[:], mu[:])
        var = pool.tile([128, 1], F32)
        nc.vector.scalar_tensor_tensor(var[:], sq1[:, 1:2], 1.0 / B, mus[:],
                                       ALU.mult, ALU.subtract)
        lnv = pool.tile([128, 1], F32)
        nc.scalar.activation(lnv[:], var[:], AF.Ln, bias=k_e1[:], scale=1.0)
        rst = pool.tile([128, 1], F32)
        nc.scalar.activation(rst[:], lnv[:], AF.Exp, bias=0.0, scale=-0.5)
        nc.vector.tensor_mul(a1v[:], rst[:], k_g1[:])
        t1 = pool.tile([128, 1], F32)
        nc.vector.tensor_mul(t1[:], mu[:], a1v[:])
        nc.vector.scalar_tensor_tensor(d1v[:], t1[:], -1.0, k_b1[:],
                                       ALU.mult, ALU.add)
        nc.vector.tensor_scalar(fpw_eff[:], k_fpw[:], a1v[:], None, ALU.mult)
        nc.gpsimd.memset(zzT[0:32, :].bitcast(F32), 0.0)
        bp = psum.tile([20, 1], F32, tag="bp")
        nc.tensor.matmul(bp[:], k_fpw[:], d1v[:], start=True, stop=True)
        nc.scalar.activation(biasEff[:], bp[:], AF.Identity, bias=k_fpb[:])
        FC2p = 2 * FC
        for d in range(NCH // 2):
            yp = psum.tile([20, FC2p], F32, tag="yp")
            for h in range(2):
                nc.tensor.matmul(yp[:, h * FC:(h + 1) * FC], fpw_eff[:],
                                 projT[:, d * FC2p + h * FC:d * FC2p + (h + 1) * FC],
                                 start=True, stop=True)
            nc.scalar.activation(yT[:, d * FC2p:(d + 1) * FC2p], yp[:],
                                 AF.Identity, bias=biasEff[:], scale=1.0,
                                 accum_out=scol3[:, d:d + 1])
            qscr2 = pool.tile([20, FC2p], BF16, tag="qscr2")
            nc.vector.scalar_tensor_tensor(
                qscr2[:], yT[:, d * FC2p:(d + 1) * FC2p], 1.0,
                yT[:, d * FC2p:(d + 1) * FC2p], ALU.mult, ALU.mult,
                accum_out=scol4[:, d:d + 1])
        nc.vector.reduce_sum(s2a[:, 0:1], scol3[:], axis=mybir.AxisListType.X)
        nc.vector.reduce_sum(s2a[:, 1:2], scol4[:], axis=mybir.AxisListType.X)
        nc.sync.dma_start(out=ar2_in[:, :], in_=s2a[:])

    with nc.semaphore("cc2") as cs:
        nc.gpsimd.collective_compute(
            "AllReduce", ALU.add, replica_groups=[list(range(NCORES))],
            ins=[ar2_in[:, :].opt()], outs=[ar2_out[:, :].opt()]).then_inc(cs, 1)
        nc.gpsimd.wait_ge(cs, 1)
        nc.all_engine_barrier()

    # ================= phase 3 =================
    with ExitStack() as ctx:
        tc = ctx.enter_context(SplitDrainTileContext(nc))
        pool = ctx.enter_context(tc.tile_pool(name="p3", bufs=3))
        psum = ctx.enter_context(tc.tile_pool(name="ps3", bufs=2, space="PSUM"))
        sq2 = pool.tile([20, 2], F32)
        nc.sync.dma_start(out=sq2[:], in_=ar2_out[:, :])
        mu2 = pool.tile([20, 1], F32)
        nc.scalar.mul(mu2[:], sq2[:, 0:1], 1.0 / B)
        mus2 = pool.tile([20, 1], F32)
        nc.vector.tensor_mul(mus2[:], mu2[:], mu2[:])
        var2 = pool.tile([20, 1], F32)
        nc.vector.scalar_tensor_tensor(var2[:], sq2[:, 1:2], 1.0 / B, mus2[:],
                                       ALU.mult, ALU.subtract)
        lnv2 = pool.tile([20, 1], F32)
        nc.scalar.activation(lnv2[:], var2[:], AF.Ln, bias=k_e2[:], scale=1.0)
        rst2 = pool.tile([20, 1], F32)
        nc.scalar.activation(rst2[:], lnv2[:], AF.Exp, bias=0.0, scale=-0.5)
        nc.vector.tensor_mul(a2v[:], rst2[:], k_g2[:])
        t2 = pool.tile([20, 1], F32)
        nc.vector.tensor_mul(t2[:], mu2[:], a2v[:])
        nc.vector.scalar_tensor_tensor(d2v[:], t2[:], -1.0, k_b2[:],
                                       ALU.mult, ALU.add)
        # 3a: z = gelu(a2*y+d2), z^2 — batched 4 chunks per op
        BL = 4 * FC
        for b in range(BS // BL):
            s = slice(b * BL, (b + 1) * BL)
            nc.scalar.activation(zzT[0:20, s], yT[:, s], AF.Gelu,
                                 bias=d2v[:], scale=a2v[:])
            nc.vector.tensor_mul(zzT[32:52, s], zzT[0:20, s], zzT[0:20, s])
        # 3b: memberships + head, pair-batched at FD=1024
        psum2 = ctx.enter_context(tc.tile_pool(name="ps3b", bufs=1, space="PSUM"))
        FC2 = 2 * FC
        for d in range(NCH // 2):
            zt = zzT[:, d * FC2:(d + 1) * FC2]
            u1 = psum.tile([128, FC2], F32, tag="u1")
            nc.tensor.matmul(u1[:, 0:FC], k_afz1[:], zt[:, 0:FC],
                             start=True, stop=True)
            nc.tensor.matmul(u1[:, FC:FC2], k_afz1[:], zt[:, FC:FC2],
                             start=True, stop=True)
            u2 = psum2.tile([72, FC2], F32, tag="u2")
            nc.tensor.matmul(u2[:, 0:FC], k_afz2[:], zt[:, 0:FC],
                             start=True, stop=True)
            nc.tensor.matmul(u2[:, FC:FC2], k_afz2[:], zt[:, FC:FC2],
                             start=True, stop=True)
            e1u = pool.tile([128, FC2], BF16, tag="e1u")
            nc.scalar.activation(e1u[:], u1[:], AF.Exp, bias=k_ub1[:], scale=-0.5)
            e1l = pool.tile([128, FC2], BF16, tag="e1l")
            nc.scalar.activation(e1l[:], u1[:], AF.Exp, bias=k_lb1[:],
                                 scale=k_ls1[:])
            e2u = pool.tile([72, FC2], BF16, tag="e2u")
            nc.scalar.activation(e2u[:], u2[:], AF.Exp, bias=k_ub2[:], scale=-0.5)
            t2l = pool.tile([72, FC2], F32, tag="t2l")
            nc.vector.tensor_scalar(t2l[:], u2[:], k_uc2[:], k_als2[:],
                                    ALU.min, ALU.mult)
            e2l = pool.tile([72, FC2], mybir.dt.int16, tag="e2l")
            nc.vector.tensor_scalar(e2l[:], t2l[:], k_ablb2[:], None, ALU.add)
            osb = pool.tile([1, FC2], F32, tag="osb")
            for h in range(2):
                hs = slice(h * FC, (h + 1) * FC)
                orow = psum.tile([1, FC], F32, tag="orow", name="orow")
                nc.tensor.matmul(orow[:], k_wh1[:], e1u[:, hs],
                                 start=True, stop=False)
                nc.tensor.matmul(orow[:], k_wh1[:], e1l[:, hs],
                                 start=False, stop=False)
                nc.tensor.matmul(orow[:], k_wh2[:], e2u[:, hs],
                                 start=False, stop=False)
                nc.tensor.matmul(orow[:], k_wh2[:], e2l[:, hs].bitcast(BF16),
                                 start=False, stop=True)
                # batch row within pair = 8p + 4h + j  (orow col = j*128 + p)
                nc.vector.tensor_scalar(
                    osb[:].rearrange("one (p s) -> one p s", s=8)[:, :, 4 * h:4 * h + 4],
                    orow[:].rearrange("one (s p) -> one p s", s=4),
                    head_b, None, ALU.add)
            ov = out[:, :].rearrange("(q s) one -> q (s one)", s=FC2)
            nc.sync.dma_start(out=ov[d:d + 1, :], in_=osb[:])
    octx.close()
    _split_multiwaits(nc)
    return nc


def _split_multiwaits(nc, max_waits=1):
    # hoist extra sync waits into single-wait nops placed just before the
    # offending instruction (walrus here rejects multi-wait instructions)
    for bb in nc.m.functions[0].blocks:
        insts = bb.instructions
        i = 0
        while i < len(insts):
            inst = insts[i]
            si = getattr(inst, "sync_info", None)
            waits = list(si.on_wait) if si and si.on_wait else []
            if len(waits) > max_waits:
                inst.sync_info = mybir.SyncInfo(
                    on_wait=waits[:max_waits], on_update=si.on_update)
                for j, w in enumerate(waits[max_waits:]):
                    n = mybir.InstNoOp(name=f"{inst.name}_ws{j}", ins=[], outs=[])
                    n.engine = inst.engine
                    n.sync_info = mybir.SyncInfo(on_wait=[w], on_update=[])
                    nc.register_instruction(n, overwrite=True)
                    insts.insert(i, n)
                    i += 1
            i += 1


LAST_RESULTS = None


def kernel(**inputs):
    global LAST_RESULTS
    import os
    x = np.asarray(inputs["x"], np.float32)
    p = {k: np.asarray(v) for k, v in inputs.items() if k != "x"}
    nc = _build(p)
    in_maps = [{"x": np.ascontiguousarray(x[i * BS:(i + 1) * BS])}
               for i in range(NCORES)]
    kw = {}
    if os.environ.get("KANFIS_TRACE") == "1":
        kw = dict(trace=True, tmpdir=os.environ.get("KANFIS_TRACE_DIR") or None)
    res = run_bass_kernel_spmd(nc, in_maps, core_ids=list(range(NCORES)), **kw)
    LAST_RESULTS = res
    return np.concatenate([res.results[i]["out"] for i in range(NCORES)], axis=0)


# revision 44
# speedup vs baseline: 1.0104x; 1.0104x over previous
"""KANFIS forward on 8 NeuronCores, data-parallel over the batch.

Per core (batch shard 16384 rows, processed as 16 chunk-pairs of 1024 so
every elementwise op runs at free-dim 1024 and amortizes engine overhead):

phase1  x (contiguous DMA; batch order permuted to col=j*128+p, undone at
        the output DMA) -> 8 PE transposes give x^T in PSUM partitions
        0:64; ACT Square writes the k=0 gaussian arg into 64:128; ONE
        packed ACT Exp produces [P ; rbf_0] (P = exp(dc*x/sigma^2), with
        per-partition scale switching function halves). The remaining 7
        RBF planes come from the squared-step DVE chain (bf16, 2x mode):
        X1=[t0;t1] via one mult + copies, PP2=[P^2;P^2], then
        X_{j+1} = X_j (.) PP2 yields plane pairs {t2,t3},{t4,t5},{t6,t7},
        where t_m = rbf_m / gamma_m and gamma folds into the baked bf16
        matmul weights. Group projection + k-sum = 4 paired bf16 matmuls
        + one float32r matmul for the linear term. BN1 partials: S1 via
        ACT accum on the PSUM->SBUF evict (applies proj bias, stores
        projT bf16), Q1 via batched ACT Square accum; per-pair partials
        land in scol columns, one DVE reduce at the end.
AR1     AllReduce [128,2]; a1 = g1*exp(-0.5*ln(var+eps)) (Ln+Exp share the
        natural_log_exp ACT table set - no table switch), d1 likewise.
phase2  BN1 affine folded into the fp weights: fpw_eff = a1 (.) fp_W^T
        (DVE), biasEff = fp_b + fp_W @ d1 (PE). Two bf16 matmuls per pair
        produce y^T [20,1024] feature-major; ACT evict applies biasEff and
        accumulates S2; DVE square-accum gives Q2.
AR2     AllReduce [20,2] -> a2,d2.
phase3  3a: z^T = Gelu(a2*y^T+d2) per-partition feature-major (no
        transposes), z^2 on DVE -> zzT [52,*] f32r (z at partitions 0:20,
        z^2 at 32:52 - DVE partition bases must be 32-aligned).
        3b: u = afz @ zz (float32r matmuls, 1 cyc/row); memberships: three
        ACT Exp -> bf16 tiles; the e2l plane instead uses a DVE int16
        bit-trick exp (bits16 = A16*clamped_exponent + B16 -> bitcast
        bf16, ~3% rel err, fine for the 2e-2 budget since the fuzzy head
        averages 400 such terms); head-sum via 4 bf16 matmuls into a
        [1,512] PSUM row per chunk, DVE adds head_b while un-permuting
        the batch order, one contiguous 4KB DMA per pair.

All parameters are baked into the NEFF as inline tensors; only x is a
runtime input. BN statistics are computed on device with two small
AllReduces. If the RBF grid is not uniformly spaced with per-group
constant sigma, a direct (non-recurrence) fallback path is built instead.

TimelineSim (cost-model) per-core estimate: ~193 us vs ~646 us for the
v0 baseline; hardware-verified rel err vs the fp64 reference: 3.8e-3.
"""
import numpy as np
import ml_dtypes
from contextlib import ExitStack

import concourse.bass as bass
import concourse.tile as tile
from concourse import mybir
from concourse.vector_clock import ScopedClock
from concourse.bass_utils import run_bass_kernel_spmd

F32 = mybir.dt.float32
F32R = mybir.dt.float32r
BF16 = mybir.dt.bfloat16
AF = mybir.ActivationFunctionType
ALU = mybir.AluOpType
BF = ml_dtypes.bfloat16

NCORES = 8
B = 131072
BS = B // NCORES          # 16384 rows per core
G, GS, K, O = 8, 8, 8, 16
TOT, R, FIN = 128, 10, 20
EPS = 1e-5
FC = 512                  # chunk free size
NCH = BS // FC            # 32 chunks


class SplitDrainTileContext(tile.TileContext):
    """walrus on this stack rejects >1 sync wait per instruction; split the
    kernel-tail drain's waits into single-wait nops."""

    def _drain_and_barrier(self, tick_clock, wait_clock):
        nc = self.nc
        nop = nc.sync.nop(nofuse=True)
        wait_clock.add_sem_waits(nop.ins, ScopedClock({None: tick_clock.global_clock}))
        si = nop.ins.sync_info
        waits = list(si.on_wait) if si and si.on_wait else []
        if len(waits) > 1:
            nop.ins.sync_info = mybir.SyncInfo(on_wait=waits[:1], on_update=si.on_update)
            for w in waits[1:]:
                n2 = nc.sync.nop(nofuse=True)
                n2.ins.sync_info = mybir.SyncInfo(on_wait=[w], on_update=[])
        nc.sync.drain()
        nc.all_engine_barrier()
        assert self.sems is not None
        popped = nc._tile_sem_poison_stack.pop()
        assert popped is self._sem_poison
        nc.clear_and_free_semaphores(list(self.sems.allocated().values()))
        nc.all_engine_barrier()


def _build(p):
    nc = bass.Bass(num_devices=NCORES)
    x = nc.dram_tensor("x", [BS, 64], F32, kind="ExternalInput")
    out = nc.dram_tensor("out", [BS, 1], F32, kind="ExternalOutput")
    ar1_in = nc.dram_tensor("ar1_in", [128, 2], F32)
    ar1_out = nc.dram_tensor("ar1_out", [128, 2], F32)
    ar2_in = nc.dram_tensor("ar2_in", [20, 2], F32)
    ar2_out = nc.dram_tensor("ar2_out", [20, 2], F32)

    # ---- baked constants (numpy) ----
    sig = np.exp(np.asarray(p["rbf_log_widths"], np.float64)) + 1e-6   # [G,K]
    cen = np.asarray(p["rbf_centres"], np.float64)                     # [G,K]
    inv = 1.0 / sig
    # recurrence structure: uniform centre spacing + k-constant sigma per group
    dcen = np.diff(cen, axis=1)
    rec_ok = (K >= 3 and np.allclose(dcen, dcen[:, :1], rtol=1e-5, atol=1e-7)
              and np.allclose(sig, sig[:, :1], rtol=1e-5, atol=1e-9))
    # seed Square scale/bias per partition.
    # Recurrence mode: one [128] vector — partitions 64:128 hold k=0 params
    # (Square input is x^T there; the squared result lands in 0:64).
    # The packed Exp then uses scale -0.5 on 0:64 (gaussian) and the P-scale
    # dc/sigma^2 on 64:128 (reads raw x^T).
    sqs = np.zeros((4, 128), np.float32)
    sqb = np.zeros((4, 128), np.float32)
    for j in range(K // 2):
        for half, k in ((0, 2 * j), (1, 2 * j + 1)):
            s = np.repeat(inv[:, k], GS)                # [64]
            b = np.repeat(-cen[:, k] * inv[:, k], GS)
            sqs[j, half * 64:(half + 1) * 64] = s
            sqb[j, half * 64:(half + 1) * 64] = b
    sqs_rec = np.zeros(128, np.float32)
    sqb_rec = np.zeros(128, np.float32)
    sqs_rec[:64] = np.repeat(inv[:, 0], GS)
    sqb_rec[:64] = np.repeat(-cen[:, 0] * inv[:, 0], GS)
    dc_g = dcen[:, 0] if rec_ok else np.zeros(G)
    exps_rec = np.zeros(128, np.float32)
    exps_rec[:64] = np.repeat(dc_g / sig[:, 0] ** 2, GS)
    exps_rec[64:] = -0.5
    # chain-plane weights: plane m is t_m = rbf_0 * P^m = rbf_m / gamma_m;
    # gamma_m = exp(-(c_m^2 - c_0^2) / (2 sigma^2)) folds into the lhsT.
    pw = np.asarray(p["proj_W"], np.float64)            # [G,O,GS]
    w = np.asarray(p["rbf_weights"], np.float64)        # [G,K]
    if rec_ok:
        gam = np.exp(-(cen ** 2 - cen[:, :1] ** 2) / (2 * sig[:, :1] ** 2))  # [G,K]
        weff = w * gam
    else:
        weff = w
    # lhsT blocks: 4 pairs {2j,2j+1} as [128,128]. In rec mode the planes are
    # t_m = rbf_m / gamma_m and gamma folds into weff.
    lhp = np.zeros((K // 2, 128, 128), np.float64)
    for j in range(K // 2):
        for half, k in ((0, 2 * j), (1, 2 * j + 1)):
            for g in range(G):
                lhp[j, half * 64 + g * GS:half * 64 + (g + 1) * GS,
                    g * O:(g + 1) * O] = pw[g].T * weff[g, k]
    linT = np.zeros((64, 128), np.float32)
    for g in range(G):
        linT[g * GS:(g + 1) * GS, g * O:(g + 1) * O] = (
            pw[g].T * np.asarray(p["rbf_linear_w"], np.float64)[g])
    pbv = np.asarray(p["proj_b"], np.float32).reshape(128, 1)
    # fuzzy layer
    su = np.exp(np.asarray(p["fz_log_su"], np.float64)) + 1e-6          # [R,FIN]
    sl = np.minimum(np.exp(np.asarray(p["fz_log_sl"], np.float64)) + 1e-6, su * 0.9)
    cz = np.asarray(p["fz_centres"], np.float64)
    # z rows at partitions 0:20, z^2 rows at 32:52 (DVE partition bases must
    # be 32-aligned, so z^2 is written at base 32; rows 20:32 stay zero)
    afz = np.zeros((52, 200), np.float32)
    for r in range(R):
        for f in range(FIN):
            m = r * FIN + f
            afz[f, m] = -2.0 * cz[r, f] / su[r, f] ** 2
            afz[32 + f, m] = 1.0 / su[r, f] ** 2
    ubias = (-0.5 * cz ** 2 / su ** 2).reshape(200, 1).astype(np.float32)
    lbias = (-0.5 * cz ** 2 / sl ** 2).reshape(200, 1).astype(np.float32)
    lscale = (-0.5 * (su / sl) ** 2).reshape(200, 1).astype(np.float32)
    wh = np.repeat(np.asarray(p["head_W"], np.float64).reshape(R, 1) * 0.5 / FIN,
                   FIN, 0).astype(np.float32)                           # [200,1]
    # bf16 bit-trick exp for the e2l plane: bits16 = A16*expnt + B16, expnt
    # clamped at -80 (below which the int16 pattern would go negative)
    A16 = 128.0 / np.log(2.0)
    B16 = 16256.0 - 486411.0 / 65536.0
    ls2 = lscale[128:, 0].astype(np.float64)
    lb2 = lbias[128:, 0].astype(np.float64)
    uc2 = ((-80.0 - lb2) / ls2).astype(np.float32).reshape(72, 1)
    als2 = (A16 * ls2).astype(np.float32).reshape(72, 1)
    ablb2 = (A16 * lb2 + B16).astype(np.float32).reshape(72, 1)
    head_b = float(np.asarray(p["head_b"]).reshape(-1)[0])

    def it(name, arr, dt=None):
        a = np.ascontiguousarray(arr)
        if dt == "bf16":
            a = a.astype(BF)
        else:
            a = a.astype(np.float32)
        return nc.inline_tensor(a, name=name)

    c_id = it("c_id", np.eye(128))
    c_sqs = it("c_sqs", sqs.T)           # [128,4]
    c_sqb = it("c_sqb", sqb.T)
    c_sqsr = it("c_sqsr", sqs_rec.reshape(128, 1))
    c_sqbr = it("c_sqbr", sqb_rec.reshape(128, 1))
    c_exsr = it("c_exsr", exps_rec.reshape(128, 1))
    c_lh = it("c_lh", lhp.transpose(1, 0, 2).reshape(128, 4 * 128), "bf16")
    c_lin = it("c_lin", linT)            # [64,128] f32
    c_pb = it("c_pb", pbv)
    c_g1 = it("c_g1", np.asarray(p["bn1_gamma"]).reshape(128, 1))
    c_b1 = it("c_b1", np.asarray(p["bn1_beta"]).reshape(128, 1))
    c_g2 = it("c_g2", np.asarray(p["bn2_gamma"]).reshape(20, 1))
    c_b2 = it("c_b2", np.asarray(p["bn2_beta"]).reshape(20, 1))
    c_fpw = it("c_fpw", np.asarray(p["fp_W"]).T)        # [128,20]
    c_fpb = it("c_fpb", np.asarray(p["fp_b"]).reshape(20, 1))
    c_afz1 = it("c_afz1", afz[:, :128])
    c_afz2 = it("c_afz2", afz[:, 128:])
    c_ub1 = it("c_ub1", ubias[:128]); c_ub2 = it("c_ub2", ubias[128:])
    c_lb1 = it("c_lb1", lbias[:128]); c_lb2 = it("c_lb2", lbias[128:])
    c_ls1 = it("c_ls1", lscale[:128]); c_ls2 = it("c_ls2", lscale[128:])
    c_wh1 = it("c_wh1", wh[:128], "bf16"); c_wh2 = it("c_wh2", wh[128:], "bf16")
    c_uc2 = it("c_uc2", uc2); c_als2 = it("c_als2", als2)
    c_ablb2 = it("c_ablb2", ablb2)

    octx = ExitStack()

    def sb(n, s, dt=F32):
        return octx.enter_context(nc.sbuf_tensor(n, s, dt))

    projT = sb("projT", [128, BS], BF16)     # 4MB persistent
    yT = sb("yT", [20, BS], BF16)
    zzT = sb("zzT", [52, BS], F32R)
    s1a = sb("s1a", [128, 2]); s2a = sb("s2a", [20, 2])
    scol1 = sb("scol1", [128, NCH // 2])     # per-pair S1 accums
    scol2 = sb("scol2", [128, NCH // 4])     # per-group Q1 accums
    scol3 = sb("scol3", [20, NCH // 2]); scol4 = sb("scol4", [20, NCH // 2])
    a1v = sb("a1v", [128, 1]); d1v = sb("d1v", [128, 1])
    a2v = sb("a2v", [20, 1]); d2v = sb("d2v", [20, 1])
    fpw_eff = sb("fpw_eff", [128, 20], BF16)
    biasEff = sb("biasEff", [20, 1])
    # const sbuf copies
    k_id = sb("k_id", [128, 128])
    k_sqs = sb("k_sqs", [128, 4]); k_sqb = sb("k_sqb", [128, 4])
    k_sqsr = sb("k_sqsr", [128, 1]); k_sqbr = sb("k_sqbr", [128, 1])
    k_exsr = sb("k_exsr", [128, 1])
    k_lh = sb("k_lh", [128, 4 * 128], BF16)
    k_lin = sb("k_lin", [64, 128], F32R); k_pb = sb("k_pb", [128, 1])
    k_g1 = sb("k_g1", [128, 1]); k_b1 = sb("k_b1", [128, 1])
    k_g2 = sb("k_g2", [20, 1]); k_b2 = sb("k_b2", [20, 1])
    k_fpw = sb("k_fpw", [128, 20]); k_fpb = sb("k_fpb", [20, 1])
    k_afz1 = sb("k_afz1", [52, 128], F32R); k_afz2 = sb("k_afz2", [52, 72], F32R)
    k_ub1 = sb("k_ub1", [128, 1]); k_ub2 = sb("k_ub2", [72, 1])
    k_lb1 = sb("k_lb1", [128, 1]); k_lb2 = sb("k_lb2", [72, 1])
    k_ls1 = sb("k_ls1", [128, 1]); k_ls2 = sb("k_ls2", [72, 1])
    k_wh1 = sb("k_wh1", [128, 1], BF16); k_wh2 = sb("k_wh2", [72, 1], BF16)
    k_uc2 = sb("k_uc2", [72, 1]); k_als2 = sb("k_als2", [72, 1])
    k_ablb2 = sb("k_ablb2", [72, 1])
    k_e1 = sb("k_e1", [128, 1]); k_e2 = sb("k_e2", [20, 1])
    k_hb4 = sb("k_hb4", [4, 1])

    # ================= phase 1 =================
    with ExitStack() as ctx:
        tc = ctx.enter_context(SplitDrainTileContext(nc))
        # consts via SWDGE in use order so they never block the x loads
        for dst, src in [(k_id, c_id), (k_sqsr, c_sqsr), (k_sqbr, c_sqbr),
                         (k_exsr, c_exsr), (k_lh, c_lh), (k_lin, c_lin),
                         (k_pb, c_pb), (k_g1, c_g1),
                         (k_b1, c_b1), (k_g2, c_g2), (k_b2, c_b2),
                         (k_fpw, c_fpw), (k_fpb, c_fpb), (k_afz1, c_afz1),
                         (k_afz2, c_afz2), (k_ub1, c_ub1), (k_ub2, c_ub2),
                         (k_lb1, c_lb1), (k_lb2, c_lb2), (k_ls1, c_ls1),
                         (k_ls2, c_ls2), (k_wh1, c_wh1), (k_wh2, c_wh2),
                         (k_uc2, c_uc2), (k_als2, c_als2),
                         (k_ablb2, c_ablb2), (k_sqs, c_sqs), (k_sqb, c_sqb)]:
            cast = dst.dtype == F32R
            nc.gpsimd.dma_start(out=dst[:],
                                in_=src[:, :].bitcast(F32R) if cast else src[:, :])
        nc.vector.memset(k_e1[:], EPS)
        nc.vector.memset(k_e2[:], EPS)
        nc.vector.memset(k_hb4[:], head_b)
        pool = ctx.enter_context(tc.tile_pool(name="p1", bufs=3))
        psum = ctx.enter_context(tc.tile_pool(name="ps1", bufs=2, space="PSUM"))
        # chunk pairs: FD=1024 per op; within a pair, column j*128+p holds
        # batch row d*1024 + 8p + j (undone at the output DMA)
        FC2 = 2 * FC
        xv = x.rearrange("(d p s) f -> d p (s f)", p=128, s=8)
        for d in range(NCH // 2):
            xt = pool.tile([128, 512], F32, tag="xt")
            nc.sync.dma_start(out=xt[:], in_=xv[d])
            uix = psum.tile([128, FC2], F32, tag="uix")
            # x^T at partitions 0:64 (transpose may only write partition 0);
            # the squared gaussian arg goes to 64:128
            for j in range(8):
                nc.tensor.transpose(uix[0:64, j * 128:(j + 1) * 128],
                                    xt[:, j * 64:(j + 1) * 64], k_id[:])
            xts = pool.tile([64, FC2], F32R, tag="xts")
            if d % 2 == 0:
                nc.scalar.copy(xts[:], uix[0:64, :])
            else:
                nc.vector.tensor_copy(xts[:], uix[0:64, :])
            pp = psum.tile([128, FC2], F32, tag="pp")
            if rec_ok:
                nc.scalar.activation(uix[64:128, :], uix[0:64, :], AF.Square,
                                     bias=k_sqbr[0:64, 0:1],
                                     scale=k_sqsr[0:64, 0:1])
                # packed exp: [P ; rbf_0]
                e0 = pool.tile([128, FC2], BF16, tag="e0")
                nc.scalar.activation(e0[:], uix[:], AF.Exp, bias=0.0,
                                     scale=k_exsr[:])
                # P duplicated to partition base 64 (DVE needs equal input
                # bases when both tensors are in SBUF)
                ptU = pool.tile([128, FC2], BF16, tag="ptU")
                nc.vector.tensor_copy(ptU[64:128, :], e0[0:64, :])
                # squared-step chain: X1=[t0;t1], PP2=[P^2;P^2],
                # X_{j+1} = X_j * PP2 gives [t2;t3],[t4;t5],[t6;t7]
                # (gamma folded into lhsT)
                eA = pool.tile([128, FC2], BF16, tag="eA")
                eB = pool.tile([128, FC2], BF16, tag="eB")
                eC = pool.tile([128, FC2], BF16, tag="eC")
                eD = pool.tile([128, FC2], BF16, tag="eD")
                pp2 = pool.tile([128, FC2], BF16, tag="pp2")
                nc.vector.tensor_copy(eA[0:64, :], e0[64:128, :])
                nc.vector.tensor_tensor(eA[64:128, :], e0[64:128, :],
                                        ptU[64:128, :], ALU.mult)
                nc.vector.tensor_tensor(pp2[0:64, :], e0[0:64, :], e0[0:64, :],
                                        ALU.mult)
                nc.vector.tensor_copy(pp2[64:128, :], pp2[0:64, :])
                nc.vector.tensor_tensor(eB[:], eA[:], pp2[:], ALU.mult)
                nc.vector.tensor_tensor(eC[:], eB[:], pp2[:], ALU.mult)
                nc.vector.tensor_tensor(eD[:], eC[:], pp2[:], ALU.mult)
                for h in range(2):
                    hs = slice(h * FC, (h + 1) * FC)
                    for j, et in enumerate([eA, eB, eC, eD]):
                        nc.tensor.matmul(pp[:, hs],
                                         k_lh[:, j * 128:(j + 1) * 128],
                                         et[:, hs], start=(j == 0), stop=False)
                    nc.tensor.matmul(pp[:, hs], k_lin[:], xts[:, hs],
                                     start=False, stop=True)
            else:
                for j in range(8):
                    nc.tensor.matmul(uix[64:128, j * 128:(j + 1) * 128],
                                     xt[:, j * 64:(j + 1) * 64], k_id[:],
                                     start=True, stop=True)
                for h in range(2):
                    hs = slice(h * FC, (h + 1) * FC)
                    for j in range(K // 2):
                        uj2 = psum.tile([128, FC], F32, tag="uj", name="uj2")
                        nc.scalar.activation(uj2[:], uix[:, hs], AF.Square,
                                             bias=k_sqb[:, j:j + 1],
                                             scale=k_sqs[:, j:j + 1])
                        ej2 = pool.tile([128, FC], BF16, tag="ej2", name="ej2")
                        nc.scalar.activation(ej2[:], uj2[:], AF.Exp,
                                             bias=0.0, scale=-0.5)
                        nc.tensor.matmul(pp[:, hs],
                                         k_lh[:, j * 128:(j + 1) * 128],
                                         ej2[:], start=(j == 0), stop=False)
                    nc.tensor.matmul(pp[:, hs], k_lin[:], xts[:, hs],
                                     start=False, stop=True)
            nc.scalar.activation(projT[:, d * FC2:(d + 1) * FC2], pp[:],
                                 AF.Identity, bias=k_pb[:], scale=1.0,
                                 accum_out=scol1[:, d:d + 1])
            if d % 2 == 1:
                qscr = pool.tile([128, 4 * FC], BF16, tag="qscr")
                g = d // 2
                nc.scalar.activation(qscr[:], projT[:, g * 4 * FC:(g + 1) * 4 * FC],
                                     AF.Square,
                                     accum_out=scol2[:, g:g + 1])
        nc.vector.reduce_sum(s1a[:, 0:1], scol1[:], axis=mybir.AxisListType.X)
        nc.vector.reduce_sum(s1a[:, 1:2], scol2[:], axis=mybir.AxisListType.X)
        nc.sync.dma_start(out=ar1_in[:, :], in_=s1a[:])

    with nc.semaphore("cc1") as cs:
        nc.gpsimd.collective_compute(
            "AllReduce", ALU.add, replica_groups=[list(range(NCORES))],
            ins=[ar1_in[:, :].opt()], outs=[ar1_out[:, :].opt()]).then_inc(cs, 1)
        nc.gpsimd.wait_ge(cs, 1)
        nc.all_engine_barrier()

    # ================= phase 2 =================
    with ExitStack() as ctx:
        tc = ctx.enter_context(SplitDrainTileContext(nc))
        pool = ctx.enter_context(tc.tile_pool(name="p2", bufs=3))
        psum = ctx.enter_context(tc.tile_pool(name="ps2", bufs=2, space="PSUM"))
        sq1 = pool.tile([128, 2], F32)
        nc.sync.dma_start(out=sq1[:], in_=ar1_out[:, :])
        mu = pool.tile([128, 1], F32)
        nc.scalar.mul(mu[:], sq1[:, 0:1], 1.0 / B)
        mus = pool.tile([128, 1], F32)
        nc.vector.tensor_mul(mus[:], mu[:], mu[:])
        var = pool.tile([128, 1], F32)
        nc.vector.scalar_tensor_tensor(var[:], sq1[:, 1:2], 1.0 / B, mus[:],
                                       ALU.mult, ALU.subtract)
        lnv = pool.tile([128, 1], F32)
        nc.scalar.activation(lnv[:], var[:], AF.Ln, bias=k_e1[:], scale=1.0)
        rst = pool.tile([128, 1], F32)
        nc.scalar.activation(rst[:], lnv[:], AF.Exp, bias=0.0, scale=-0.5)
        nc.vector.tensor_mul(a1v[:], rst[:], k_g1[:])
        t1 = pool.tile([128, 1], F32)
        nc.vector.tensor_mul(t1[:], mu[:], a1v[:])
        nc.vector.scalar_tensor_tensor(d1v[:], t1[:], -1.0, k_b1[:],
                                       ALU.mult, ALU.add)
        nc.vector.tensor_scalar(fpw_eff[:], k_fpw[:], a1v[:], None, ALU.mult)
        nc.gpsimd.memset(zzT[0:32, :].bitcast(F32), 0.0)
        bp = psum.tile([20, 1], F32, tag="bp")
        nc.tensor.matmul(bp[:], k_fpw[:], d1v[:], start=True, stop=True)
        nc.scalar.activation(biasEff[:], bp[:], AF.Identity, bias=k_fpb[:])
        FC2p = 2 * FC
        for d in range(NCH // 2):
            yp = psum.tile([20, FC2p], F32, tag="yp")
            for h in range(2):
                nc.tensor.matmul(yp[:, h * FC:(h + 1) * FC], fpw_eff[:],
                                 projT[:, d * FC2p + h * FC:d * FC2p + (h + 1) * FC],
                                 start=True, stop=True)
            nc.scalar.activation(yT[:, d * FC2p:(d + 1) * FC2p], yp[:],
                                 AF.Identity, bias=biasEff[:], scale=1.0,
                                 accum_out=scol3[:, d:d + 1])
            qscr2 = pool.tile([20, FC2p], BF16, tag="qscr2")
            nc.vector.scalar_tensor_tensor(
                qscr2[:], yT[:, d * FC2p:(d + 1) * FC2p], 1.0,
                yT[:, d * FC2p:(d + 1) * FC2p], ALU.mult, ALU.mult,
                accum_out=scol4[:, d:d + 1])
        nc.vector.reduce_sum(s2a[:, 0:1], scol3[:], axis=mybir.AxisListType.X)
        nc.vector.reduce_sum(s2a[:, 1:2], scol4[:], axis=mybir.AxisListType.X)
        nc.sync.dma_start(out=ar2_in[:, :], in_=s2a[:])

    with nc.semaphore("cc2") as cs:
        nc.gpsimd.collective_compute(
            "AllReduce", ALU.add, replica_groups=[list(range(NCORES))],
            ins=[ar2_in[:, :].opt()], outs=[ar2_out[:, :].opt()]).then_inc(cs, 1)
        nc.gpsimd.wait_ge(cs, 1)
        nc.all_engine_barrier()

    # ================= phase 3 =================
    with ExitStack() as ctx:
        tc = ctx.enter_context(SplitDrainTileContext(nc))
        pool = ctx.enter_context(tc.tile_pool(name="p3", bufs=3))
        psum = ctx.enter_context(tc.tile_pool(name="ps3", bufs=2, space="PSUM"))
        sq2 = pool.tile([20, 2], F32)
        nc.sync.dma_start(out=sq2[:], in_=ar2_out[:, :])
        mu2 = pool.tile([20, 1], F32)
        nc.scalar.mul(mu2[:], sq2[:, 0:1], 1.0 / B)
        mus2 = pool.tile([20, 1], F32)
        nc.vector.tensor_mul(mus2[:], mu2[:], mu2[:])
        var2 = pool.tile([20, 1], F32)
        nc.vector.scalar_tensor_tensor(var2[:], sq2[:, 1:2], 1.0 / B, mus2[:],
                                       ALU.mult, ALU.subtract)
        lnv2 = pool.tile([20, 1], F32)
        nc.scalar.activation(lnv2[:], var2[:], AF.Ln, bias=k_e2[:], scale=1.0)
        rst2 = pool.tile([20, 1], F32)
        nc.scalar.activation(rst2[:], lnv2[:], AF.Exp, bias=0.0, scale=-0.5)
        nc.vector.tensor_mul(a2v[:], rst2[:], k_g2[:])
        t2 = pool.tile([20, 1], F32)
        nc.vector.tensor_mul(t2[:], mu2[:], a2v[:])
        nc.vector.scalar_tensor_tensor(d2v[:], t2[:], -1.0, k_b2[:],
                                       ALU.mult, ALU.add)
        # 3a: z = gelu(a2*y+d2), z^2 — batched 4 chunks per op
        BL = 4 * FC
        for b in range(BS // BL):
            s = slice(b * BL, (b + 1) * BL)
            nc.scalar.activation(zzT[0:20, s], yT[:, s], AF.Gelu,
                                 bias=d2v[:], scale=a2v[:])
            nc.vector.tensor_mul(zzT[32:52, s], zzT[0:20, s], zzT[0:20, s])
        # 3b: memberships + head, pair-batched at FD=1024
        psum2 = ctx.enter_context(tc.tile_pool(name="ps3b", bufs=1, space="PSUM"))
        FC2 = 2 * FC
        for d in range(NCH // 2):
            zt = zzT[:, d * FC2:(d + 1) * FC2]
            u1 = psum.tile([128, FC2], F32, tag="u1")
            nc.tensor.matmul(u1[:, 0:FC], k_afz1[:], zt[:, 0:FC],
                             start=True, stop=True)
            nc.tensor.matmul(u1[:, FC:FC2], k_afz1[:], zt[:, FC:FC2],
                             start=True, stop=True)
            u2 = psum2.tile([72, FC2], F32, tag="u2")
            nc.tensor.matmul(u2[:, 0:FC], k_afz2[:], zt[:, 0:FC],
                             start=True, stop=True)
            nc.tensor.matmul(u2[:, FC:FC2], k_afz2[:], zt[:, FC:FC2],
                             start=True, stop=True)
            e1u = pool.tile([128, FC2], BF16, tag="e1u")
            nc.scalar.activation(e1u[:], u1[:], AF.Exp, bias=k_ub1[:], scale=-0.5)
            e1l = pool.tile([128, FC2], BF16, tag="e1l")
            nc.scalar.activation(e1l[:], u1[:], AF.Exp, bias=k_lb1[:],
                                 scale=k_ls1[:])
            e2u = pool.tile([72, FC2], BF16, tag="e2u")
            nc.scalar.activation(e2u[:], u2[:], AF.Exp, bias=k_ub2[:], scale=-0.5)
            t2l = pool.tile([72, FC2], F32, tag="t2l")
            nc.vector.tensor_scalar(t2l[:], u2[:], k_uc2[:], k_als2[:],
                                    ALU.min, ALU.mult)
            e2l = pool.tile([72, FC2], mybir.dt.int16, tag="e2l")
            nc.vector.tensor_scalar(e2l[:], t2l[:], k_ablb2[:], None, ALU.add)
            osb = pool.tile([1, FC2], F32, tag="osb")
            orow = psum2.tile([1, FC2], F32, tag="orow")
            for h in range(2):
                hs = slice(h * FC, (h + 1) * FC)
                ohs = orow[:, h * FC:(h + 1) * FC]
                nc.tensor.matmul(ohs, k_wh1[:], e1u[:, hs],
                                 start=True, stop=False)
                nc.tensor.matmul(ohs, k_wh1[:], e1l[:, hs],
                                 start=False, stop=False)
                nc.tensor.matmul(ohs, k_wh2[:], e2u[:, hs],
                                 start=False, stop=False)
                nc.tensor.matmul(ohs, k_wh2[:], e2l[:, hs].bitcast(BF16),
                                 start=False, stop=True)
            # batch row within pair = 8p + 4h + j  (orow col = h*512+j*128+p)
            nc.vector.tensor_scalar(
                osb[:].rearrange("one (p h j) -> one p h j", h=2, j=4),
                orow[:].rearrange("one (h j p) -> one p h j", h=2, j=4),
                head_b, None, ALU.add)
            ov = out[:, :].rearrange("(q s) one -> q (s one)", s=FC2)
            nc.sync.dma_start(out=ov[d:d + 1, :], in_=osb[:])
    octx.close()
    _split_multiwaits(nc)
    return nc


def _split_multiwaits(nc, max_waits=1):
    # hoist extra sync waits into single-wait nops placed just before the
    # offending instruction (walrus here rejects multi-wait instructions)
    for bb in nc.m.functions[0].blocks:
        insts = bb.instructions
        i = 0
        while i < len(insts):
            inst = insts[i]
            si = getattr(inst, "sync_info", None)
            waits = list(si.on_wait) if si and si.on_wait else []
            if len(waits) > max_waits:
                inst.sync_info = mybir.SyncInfo(
                    on_wait=waits[:max_waits], on_update=si.on_update)
                for j, w in enumerate(waits[max_waits:]):
                    n = mybir.InstNoOp(name=f"{inst.name}_ws{j}", ins=[], outs=[])
                    n.engine = inst.engine
                    n.sync_info = mybir.SyncInfo(on_wait=[w], on_update=[])
                    nc.register_instruction(n, overwrite=True)
                    insts.insert(i, n)
                    i += 1
            i += 1


LAST_RESULTS = None


def kernel(**inputs):
    global LAST_RESULTS
    import os
    x = np.asarray(inputs["x"], np.float32)
    p = {k: np.asarray(v) for k, v in inputs.items() if k != "x"}
    nc = _build(p)
    in_maps = [{"x": np.ascontiguousarray(x[i * BS:(i + 1) * BS])}
               for i in range(NCORES)]
    kw = {}
    if os.environ.get("KANFIS_TRACE") == "1":
        kw = dict(trace=True, tmpdir=os.environ.get("KANFIS_TRACE_DIR") or None)
    res = run_bass_kernel_spmd(nc, in_maps, core_ids=list(range(NCORES)), **kw)
    LAST_RESULTS = res
    return np.concatenate([res.results[i]["out"] for i in range(NCORES)], axis=0)


# revision 48
# speedup vs baseline: 1.0131x; 1.0027x over previous
"""KANFIS forward on 8 NeuronCores, data-parallel over the batch.

Per core (batch shard 16384 rows, processed as 16 chunk-pairs of 1024 so
every elementwise op runs at free-dim 1024 and amortizes engine overhead):

phase1  x (contiguous DMA; batch order permuted to col=j*128+p, undone at
        the output DMA) -> 8 PE transposes give x^T in PSUM partitions
        0:64; ACT Square writes the k=0 gaussian arg into 64:128; ONE
        packed ACT Exp produces [P ; rbf_0] (P = exp(dc*x/sigma^2), with
        per-partition scale switching function halves). The remaining 7
        RBF planes come from the squared-step DVE chain (bf16, 2x mode):
        X1=[t0;t1] via one mult + copies, PP2=[P^2;P^2], then
        X_{j+1} = X_j (.) PP2 yields plane pairs {t2,t3},{t4,t5},{t6,t7},
        where t_m = rbf_m / gamma_m and gamma folds into the baked bf16
        matmul weights. Group projection + k-sum = 4 paired bf16 matmuls
        + one float32r matmul for the linear term. BN1 partials: S1 via
        ACT accum on the PSUM->SBUF evict (applies proj bias, stores
        projT bf16), Q1 via batched ACT Square accum; per-pair partials
        land in scol columns, one DVE reduce at the end.
AR1     AllReduce [128,2]; a1 = g1*exp(-0.5*ln(var+eps)) (Ln+Exp share the
        natural_log_exp ACT table set - no table switch), d1 likewise.
phase2  BN1 affine folded into the fp weights: fpw_eff = a1 (.) fp_W^T
        (DVE), biasEff = fp_b + fp_W @ d1 (PE). Two bf16 matmuls per pair
        produce y^T [20,1024] feature-major; ACT evict applies biasEff and
        accumulates S2; DVE square-accum gives Q2.
AR2     AllReduce [20,2] -> a2,d2.
phase3  3a: z^T = Gelu(a2*y^T+d2) per-partition feature-major (no
        transposes), z^2 on DVE -> zzT [52,*] f32r (z at partitions 0:20,
        z^2 at 32:52 - DVE partition bases must be 32-aligned).
        3b: u = afz @ zz (float32r matmuls, 1 cyc/row); memberships: three
        ACT Exp -> bf16 tiles; the e2l plane instead uses a DVE int16
        bit-trick exp (bits16 = A16*clamped_exponent + B16 -> bitcast
        bf16, ~3% rel err, fine for the 2e-2 budget since the fuzzy head
        averages 400 such terms); head-sum via 8 bf16 matmuls into a
        two-bank [1,1024] PSUM row per pair, one DVE op adds head_b while
        un-permuting the batch order, one contiguous 4KB DMA per pair.

All parameters are baked into the NEFF as inline tensors; only x is a
runtime input. BN statistics are computed on device with two small
AllReduces. If the RBF grid is not uniformly spaced with per-group
constant sigma, a direct (non-recurrence) fallback path is built instead.

TimelineSim (cost-model) per-core estimate: ~191 us vs ~646 us for the
v0 baseline; hardware-verified rel err vs the fp64 reference: 3.8e-3.
"""
import numpy as np
import ml_dtypes
from contextlib import ExitStack

import concourse.bass as bass
import concourse.tile as tile
from concourse import mybir
from concourse.vector_clock import ScopedClock
from concourse.bass_utils import run_bass_kernel_spmd

F32 = mybir.dt.float32
F32R = mybir.dt.float32r
BF16 = mybir.dt.bfloat16
AF = mybir.ActivationFunctionType
ALU = mybir.AluOpType
BF = ml_dtypes.bfloat16

NCORES = 8
B = 131072
BS = B // NCORES          # 16384 rows per core
G, GS, K, O = 8, 8, 8, 16
TOT, R, FIN = 128, 10, 20
EPS = 1e-5
FC = 512                  # chunk free size
NCH = BS // FC            # 32 chunks


class SplitDrainTileContext(tile.TileContext):
    """walrus on this stack rejects >1 sync wait per instruction; split the
    kernel-tail drain's waits into single-wait nops."""

    def _drain_and_barrier(self, tick_clock, wait_clock):
        nc = self.nc
        nop = nc.sync.nop(nofuse=True)
        wait_clock.add_sem_waits(nop.ins, ScopedClock({None: tick_clock.global_clock}))
        si = nop.ins.sync_info
        waits = list(si.on_wait) if si and si.on_wait else []
        if len(waits) > 1:
            nop.ins.sync_info = mybir.SyncInfo(on_wait=waits[:1], on_update=si.on_update)
            for w in waits[1:]:
                n2 = nc.sync.nop(nofuse=True)
                n2.ins.sync_info = mybir.SyncInfo(on_wait=[w], on_update=[])
        nc.sync.drain()
        nc.all_engine_barrier()
        assert self.sems is not None
        popped = nc._tile_sem_poison_stack.pop()
        assert popped is self._sem_poison
        nc.clear_and_free_semaphores(list(self.sems.allocated().values()))
        nc.all_engine_barrier()


def _build(p):
    nc = bass.Bass(num_devices=NCORES)
    x = nc.dram_tensor("x", [BS, 64], F32, kind="ExternalInput")
    out = nc.dram_tensor("out", [BS, 1], F32, kind="ExternalOutput")
    ar1_in = nc.dram_tensor("ar1_in", [128, 2], F32)
    ar1_out = nc.dram_tensor("ar1_out", [128, 2], F32)
    ar2_in = nc.dram_tensor("ar2_in", [20, 2], F32)
    ar2_out = nc.dram_tensor("ar2_out", [20, 2], F32)

    # ---- baked constants (numpy) ----
    sig = np.exp(np.asarray(p["rbf_log_widths"], np.float64)) + 1e-6   # [G,K]
    cen = np.asarray(p["rbf_centres"], np.float64)                     # [G,K]
    inv = 1.0 / sig
    # recurrence structure: uniform centre spacing + k-constant sigma per group
    dcen = np.diff(cen, axis=1)
    rec_ok = (K >= 3 and np.allclose(dcen, dcen[:, :1], rtol=1e-5, atol=1e-7)
              and np.allclose(sig, sig[:, :1], rtol=1e-5, atol=1e-9))
    # seed Square scale/bias per partition.
    # Recurrence mode: one [128] vector — partitions 64:128 hold k=0 params
    # (Square input is x^T there; the squared result lands in 0:64).
    # The packed Exp then uses scale -0.5 on 0:64 (gaussian) and the P-scale
    # dc/sigma^2 on 64:128 (reads raw x^T).
    sqs = np.zeros((4, 128), np.float32)
    sqb = np.zeros((4, 128), np.float32)
    for j in range(K // 2):
        for half, k in ((0, 2 * j), (1, 2 * j + 1)):
            s = np.repeat(inv[:, k], GS)                # [64]
            b = np.repeat(-cen[:, k] * inv[:, k], GS)
            sqs[j, half * 64:(half + 1) * 64] = s
            sqb[j, half * 64:(half + 1) * 64] = b
    sqs_rec = np.zeros(128, np.float32)
    sqb_rec = np.zeros(128, np.float32)
    sqs_rec[:64] = np.repeat(inv[:, 0], GS)
    sqb_rec[:64] = np.repeat(-cen[:, 0] * inv[:, 0], GS)
    dc_g = dcen[:, 0] if rec_ok else np.zeros(G)
    exps_rec = np.zeros(128, np.float32)
    exps_rec[:64] = np.repeat(dc_g / sig[:, 0] ** 2, GS)
    exps_rec[64:] = -0.5
    # chain-plane weights: plane m is t_m = rbf_0 * P^m = rbf_m / gamma_m;
    # gamma_m = exp(-(c_m^2 - c_0^2) / (2 sigma^2)) folds into the lhsT.
    pw = np.asarray(p["proj_W"], np.float64)            # [G,O,GS]
    w = np.asarray(p["rbf_weights"], np.float64)        # [G,K]
    if rec_ok:
        gam = np.exp(-(cen ** 2 - cen[:, :1] ** 2) / (2 * sig[:, :1] ** 2))  # [G,K]
        weff = w * gam
    else:
        weff = w
    # lhsT blocks: 4 pairs {2j,2j+1} as [128,128]. In rec mode the planes are
    # t_m = rbf_m / gamma_m and gamma folds into weff.
    lhp = np.zeros((K // 2, 128, 128), np.float64)
    for j in range(K // 2):
        for half, k in ((0, 2 * j), (1, 2 * j + 1)):
            for g in range(G):
                lhp[j, half * 64 + g * GS:half * 64 + (g + 1) * GS,
                    g * O:(g + 1) * O] = pw[g].T * weff[g, k]
    linT = np.zeros((64, 128), np.float32)
    for g in range(G):
        linT[g * GS:(g + 1) * GS, g * O:(g + 1) * O] = (
            pw[g].T * np.asarray(p["rbf_linear_w"], np.float64)[g])
    pbv = np.asarray(p["proj_b"], np.float32).reshape(128, 1)
    # fuzzy layer
    su = np.exp(np.asarray(p["fz_log_su"], np.float64)) + 1e-6          # [R,FIN]
    sl = np.minimum(np.exp(np.asarray(p["fz_log_sl"], np.float64)) + 1e-6, su * 0.9)
    cz = np.asarray(p["fz_centres"], np.float64)
    # z rows at partitions 0:20, z^2 rows at 32:52 (DVE partition bases must
    # be 32-aligned, so z^2 is written at base 32; rows 20:32 stay zero)
    afz = np.zeros((52, 200), np.float32)
    for r in range(R):
        for f in range(FIN):
            m = r * FIN + f
            afz[f, m] = -2.0 * cz[r, f] / su[r, f] ** 2
            afz[32 + f, m] = 1.0 / su[r, f] ** 2
    ubias = (-0.5 * cz ** 2 / su ** 2).reshape(200, 1).astype(np.float32)
    lbias = (-0.5 * cz ** 2 / sl ** 2).reshape(200, 1).astype(np.float32)
    lscale = (-0.5 * (su / sl) ** 2).reshape(200, 1).astype(np.float32)
    wh = np.repeat(np.asarray(p["head_W"], np.float64).reshape(R, 1) * 0.5 / FIN,
                   FIN, 0).astype(np.float32)                           # [200,1]
    # bf16 bit-trick exp for the e2l plane: bits16 = A16*expnt + B16, expnt
    # clamped at -80 (below which the int16 pattern would go negative)
    A16 = 128.0 / np.log(2.0)
    B16 = 16256.0 - 486411.0 / 65536.0
    ls2 = lscale[128:, 0].astype(np.float64)
    lb2 = lbias[128:, 0].astype(np.float64)
    uc2 = ((-80.0 - lb2) / ls2).astype(np.float32).reshape(72, 1)
    als2 = (A16 * ls2).astype(np.float32).reshape(72, 1)
    ablb2 = (A16 * lb2 + B16).astype(np.float32).reshape(72, 1)
    head_b = float(np.asarray(p["head_b"]).reshape(-1)[0])

    def it(name, arr, dt=None):
        a = np.ascontiguousarray(arr)
        if dt == "bf16":
            a = a.astype(BF)
        else:
            a = a.astype(np.float32)
        return nc.inline_tensor(a, name=name)

    c_id = it("c_id", np.eye(128))
    c_sqs = it("c_sqs", sqs.T)           # [128,4]
    c_sqb = it("c_sqb", sqb.T)
    c_sqsr = it("c_sqsr", sqs_rec.reshape(128, 1))
    c_sqbr = it("c_sqbr", sqb_rec.reshape(128, 1))
    c_exsr = it("c_exsr", exps_rec.reshape(128, 1))
    c_lh = it("c_lh", lhp.transpose(1, 0, 2).reshape(128, 4 * 128), "bf16")
    c_lin = it("c_lin", linT)            # [64,128] f32
    c_pb = it("c_pb", pbv)
    c_g1 = it("c_g1", np.asarray(p["bn1_gamma"]).reshape(128, 1))
    c_b1 = it("c_b1", np.asarray(p["bn1_beta"]).reshape(128, 1))
    c_g2 = it("c_g2", np.asarray(p["bn2_gamma"]).reshape(20, 1))
    c_b2 = it("c_b2", np.asarray(p["bn2_beta"]).reshape(20, 1))
    c_fpw = it("c_fpw", np.asarray(p["fp_W"]).T)        # [128,20]
    c_fpb = it("c_fpb", np.asarray(p["fp_b"]).reshape(20, 1))
    c_afz1 = it("c_afz1", afz[:, :128])
    c_afz2 = it("c_afz2", afz[:, 128:])
    c_ub1 = it("c_ub1", ubias[:128]); c_ub2 = it("c_ub2", ubias[128:])
    c_lb1 = it("c_lb1", lbias[:128]); c_lb2 = it("c_lb2", lbias[128:])
    c_ls1 = it("c_ls1", lscale[:128]); c_ls2 = it("c_ls2", lscale[128:])
    c_wh1 = it("c_wh1", wh[:128], "bf16"); c_wh2 = it("c_wh2", wh[128:], "bf16")
    c_uc2 = it("c_uc2", uc2); c_als2 = it("c_als2", als2)
    c_ablb2 = it("c_ablb2", ablb2)

    octx = ExitStack()

    def sb(n, s, dt=F32):
        return octx.enter_context(nc.sbuf_tensor(n, s, dt))

    projT = sb("projT", [128, BS], BF16)     # 4MB persistent
    yT = sb("yT", [20, BS], BF16)
    zzT = sb("zzT", [52, BS], F32R)
    s1a = sb("s1a", [128, 2]); s2a = sb("s2a", [20, 2])
    scol1 = sb("scol1", [128, NCH // 2])     # per-pair S1 accums
    scol2 = sb("scol2", [128, NCH // 4])     # per-group Q1 accums
    scol3 = sb("scol3", [20, NCH // 2]); scol4 = sb("scol4", [20, NCH // 2])
    a1v = sb("a1v", [128, 1]); d1v = sb("d1v", [128, 1])
    a2v = sb("a2v", [20, 1]); d2v = sb("d2v", [20, 1])
    fpw_eff = sb("fpw_eff", [128, 20], BF16)
    biasEff = sb("biasEff", [20, 1])
    # const sbuf copies
    k_id = sb("k_id", [128, 128])
    k_sqs = sb("k_sqs", [128, 4]); k_sqb = sb("k_sqb", [128, 4])
    k_sqsr = sb("k_sqsr", [128, 1]); k_sqbr = sb("k_sqbr", [128, 1])
    k_exsr = sb("k_exsr", [128, 1])
    k_lh = sb("k_lh", [128, 4 * 128], BF16)
    k_lin = sb("k_lin", [64, 128], F32R); k_pb = sb("k_pb", [128, 1])
    k_g1 = sb("k_g1", [128, 1]); k_b1 = sb("k_b1", [128, 1])
    k_g2 = sb("k_g2", [20, 1]); k_b2 = sb("k_b2", [20, 1])
    k_fpw = sb("k_fpw", [128, 20]); k_fpb = sb("k_fpb", [20, 1])
    k_afz1 = sb("k_afz1", [52, 128], F32R); k_afz2 = sb("k_afz2", [52, 72], F32R)
    k_ub1 = sb("k_ub1", [128, 1]); k_ub2 = sb("k_ub2", [72, 1])
    k_lb1 = sb("k_lb1", [128, 1]); k_lb2 = sb("k_lb2", [72, 1])
    k_ls1 = sb("k_ls1", [128, 1]); k_ls2 = sb("k_ls2", [72, 1])
    k_wh1 = sb("k_wh1", [128, 1], BF16); k_wh2 = sb("k_wh2", [72, 1], BF16)
    k_uc2 = sb("k_uc2", [72, 1]); k_als2 = sb("k_als2", [72, 1])
    k_ablb2 = sb("k_ablb2", [72, 1])
    k_e1 = sb("k_e1", [128, 1]); k_e2 = sb("k_e2", [20, 1])
    k_hb4 = sb("k_hb4", [4, 1])

    # ================= phase 1 =================
    with ExitStack() as ctx:
        tc = ctx.enter_context(SplitDrainTileContext(nc))
        # consts via SWDGE in use order so they never block the x loads
        for dst, src in [(k_id, c_id), (k_sqsr, c_sqsr), (k_sqbr, c_sqbr),
                         (k_exsr, c_exsr), (k_lh, c_lh), (k_lin, c_lin),
                         (k_pb, c_pb), (k_g1, c_g1),
                         (k_b1, c_b1), (k_g2, c_g2), (k_b2, c_b2),
                         (k_fpw, c_fpw), (k_fpb, c_fpb), (k_afz1, c_afz1),
                         (k_afz2, c_afz2), (k_ub1, c_ub1), (k_ub2, c_ub2),
                         (k_lb1, c_lb1), (k_lb2, c_lb2), (k_ls1, c_ls1),
                         (k_ls2, c_ls2), (k_wh1, c_wh1), (k_wh2, c_wh2),
                         (k_uc2, c_uc2), (k_als2, c_als2),
                         (k_ablb2, c_ablb2), (k_sqs, c_sqs), (k_sqb, c_sqb)]:
            cast = dst.dtype == F32R
            nc.gpsimd.dma_start(out=dst[:],
                                in_=src[:, :].bitcast(F32R) if cast else src[:, :])
        nc.vector.memset(k_e1[:], EPS)
        nc.vector.memset(k_e2[:], EPS)
        nc.vector.memset(k_hb4[:], head_b)
        pool = ctx.enter_context(tc.tile_pool(name="p1", bufs=3))
        psum = ctx.enter_context(tc.tile_pool(name="ps1", bufs=2, space="PSUM"))
        # PE clock warm-up: the HAM gate keeps the PE at half clock until
        # ~3.4us of sustained activity; burn that window on dummy matmuls
        # (garbage data, result never read, slot shared with the pp tag)
        warm = psum.tile([1, 2 * FC], F32, tag="pp", name="warm")
        for i in range(3):
            nc.tensor.matmul(warm[0:1, 0:FC], k_e1[0:52, 0:1],
                             zzT[:, i * FC:(i + 1) * FC].bitcast(F32),
                             start=True, stop=True)
        # chunk pairs: FD=1024 per op; within a pair, column j*128+p holds
        # batch row d*1024 + 8p + j (undone at the output DMA)
        FC2 = 2 * FC
        xv = x.rearrange("(d p s) f -> d p (s f)", p=128, s=8)
        for d in range(NCH // 2):
            xt = pool.tile([128, 512], F32, tag="xt")
            nc.sync.dma_start(out=xt[:], in_=xv[d])
            uix = psum.tile([128, FC2], F32, tag="uix")
            # x^T at partitions 0:64 (transpose may only write partition 0);
            # the squared gaussian arg goes to 64:128
            for j in range(8):
                nc.tensor.transpose(uix[0:64, j * 128:(j + 1) * 128],
                                    xt[:, j * 64:(j + 1) * 64], k_id[:])
            xts = pool.tile([64, FC2], F32R, tag="xts")
            if d % 2 == 0:
                nc.scalar.copy(xts[:], uix[0:64, :])
            else:
                nc.vector.tensor_copy(xts[:], uix[0:64, :])
            pp = psum.tile([128, FC2], F32, tag="pp")
            if rec_ok:
                nc.scalar.activation(uix[64:128, :], uix[0:64, :], AF.Square,
                                     bias=k_sqbr[0:64, 0:1],
                                     scale=k_sqsr[0:64, 0:1])
                # packed exp: [P ; rbf_0]
                e0 = pool.tile([128, FC2], BF16, tag="e0")
                nc.scalar.activation(e0[:], uix[:], AF.Exp, bias=0.0,
                                     scale=k_exsr[:])
                # P duplicated to partition base 64 (DVE needs equal input
                # bases when both tensors are in SBUF)
                ptU = pool.tile([128, FC2], BF16, tag="ptU")
                nc.vector.tensor_copy(ptU[64:128, :], e0[0:64, :])
                # squared-step chain: X1=[t0;t1], PP2=[P^2;P^2],
                # X_{j+1} = X_j * PP2 gives [t2;t3],[t4;t5],[t6;t7]
                # (gamma folded into lhsT)
                eA = pool.tile([128, FC2], BF16, tag="eA")
                eB = pool.tile([128, FC2], BF16, tag="eB")
                eC = pool.tile([128, FC2], BF16, tag="eC")
                eD = pool.tile([128, FC2], BF16, tag="eD")
                pp2 = pool.tile([128, FC2], BF16, tag="pp2")
                nc.vector.tensor_copy(eA[0:64, :], e0[64:128, :])
                nc.vector.tensor_tensor(eA[64:128, :], e0[64:128, :],
                                        ptU[64:128, :], ALU.mult)
                nc.vector.tensor_tensor(pp2[0:64, :], e0[0:64, :], e0[0:64, :],
                                        ALU.mult)
                nc.vector.tensor_copy(pp2[64:128, :], pp2[0:64, :])
                nc.vector.tensor_tensor(eB[:], eA[:], pp2[:], ALU.mult)
                nc.vector.tensor_tensor(eC[:], eB[:], pp2[:], ALU.mult)
                nc.vector.tensor_tensor(eD[:], eC[:], pp2[:], ALU.mult)
                for h in range(2):
                    hs = slice(h * FC, (h + 1) * FC)
                    for j, et in enumerate([eA, eB, eC, eD]):
                        nc.tensor.matmul(pp[:, hs],
                                         k_lh[:, j * 128:(j + 1) * 128],
                                         et[:, hs], start=(j == 0), stop=False)
                    nc.tensor.matmul(pp[:, hs], k_lin[:], xts[:, hs],
                                     start=False, stop=True)
            else:
                for j in range(8):
                    nc.tensor.matmul(uix[64:128, j * 128:(j + 1) * 128],
                                     xt[:, j * 64:(j + 1) * 64], k_id[:],
                                     start=True, stop=True)
                for h in range(2):
                    hs = slice(h * FC, (h + 1) * FC)
                    for j in range(K // 2):
                        uj2 = psum.tile([128, FC], F32, tag="uj", name="uj2")
                        nc.scalar.activation(uj2[:], uix[:, hs], AF.Square,
                                             bias=k_sqb[:, j:j + 1],
                                             scale=k_sqs[:, j:j + 1])
                        ej2 = pool.tile([128, FC], BF16, tag="ej2", name="ej2")
                        nc.scalar.activation(ej2[:], uj2[:], AF.Exp,
                                             bias=0.0, scale=-0.5)
                        nc.tensor.matmul(pp[:, hs],
                                         k_lh[:, j * 128:(j + 1) * 128],
                                         ej2[:], start=(j == 0), stop=False)
                    nc.tensor.matmul(pp[:, hs], k_lin[:], xts[:, hs],
                                     start=False, stop=True)
            nc.scalar.activation(projT[:, d * FC2:(d + 1) * FC2], pp[:],
                                 AF.Identity, bias=k_pb[:], scale=1.0,
                                 accum_out=scol1[:, d:d + 1])
            if d % 2 == 1:
                qscr = pool.tile([128, 4 * FC], BF16, tag="qscr")
                g = d // 2
                nc.scalar.activation(qscr[:], projT[:, g * 4 * FC:(g + 1) * 4 * FC],
                                     AF.Square,
                                     accum_out=scol2[:, g:g + 1])
        nc.vector.reduce_sum(s1a[:, 0:1], scol1[:], axis=mybir.AxisListType.X)
        nc.vector.reduce_sum(s1a[:, 1:2], scol2[:], axis=mybir.AxisListType.X)
        nc.sync.dma_start(out=ar1_in[:, :], in_=s1a[:])

    with nc.semaphore("cc1") as cs:
        nc.gpsimd.collective_compute(
            "AllReduce", ALU.add, replica_groups=[list(range(NCORES))],
            ins=[ar1_in[:, :].opt()], outs=[ar1_out[:, :].opt()]).then_inc(cs, 1)
        nc.gpsimd.wait_ge(cs, 1)
        nc.all_engine_barrier()

    # ================= phase 2 =================
    with ExitStack() as ctx:
        tc = ctx.enter_context(SplitDrainTileContext(nc))
        pool = ctx.enter_context(tc.tile_pool(name="p2", bufs=3))
        psum = ctx.enter_context(tc.tile_pool(name="ps2", bufs=2, space="PSUM"))
        sq1 = pool.tile([128, 2], F32)
        nc.sync.dma_start(out=sq1[:], in_=ar1_out[:, :])
        mu = pool.tile([128, 1], F32)
        nc.scalar.mul(mu[:], sq1[:, 0:1], 1.0 / B)
        mus = pool.tile([128, 1], F32)
        nc.vector.tensor_mul(mus[:], mu[:], mu[:])
        var = pool.tile([128, 1], F32)
        nc.vector.scalar_tensor_tensor(var[:], sq1[:, 1:2], 1.0 / B, mus[:],
                                       ALU.mult, ALU.subtract)
        lnv = pool.tile([128, 1], F32)
        nc.scalar.activation(lnv[:], var[:], AF.Ln, bias=k_e1[:], scale=1.0)
        rst = pool.tile([128, 1], F32)
        nc.scalar.activation(rst[:], lnv[:], AF.Exp, bias=0.0, scale=-0.5)
        nc.vector.tensor_mul(a1v[:], rst[:], k_g1[:])
        t1 = pool.tile([128, 1], F32)
        nc.vector.tensor_mul(t1[:], mu[:], a1v[:])
        nc.vector.scalar_tensor_tensor(d1v[:], t1[:], -1.0, k_b1[:],
                                       ALU.mult, ALU.add)
        nc.vector.tensor_scalar(fpw_eff[:], k_fpw[:], a1v[:], None, ALU.mult)
        nc.gpsimd.memset(zzT[0:32, :].bitcast(F32), 0.0)
        bp = psum.tile([20, 1], F32, tag="bp")
        nc.tensor.matmul(bp[:], k_fpw[:], d1v[:], start=True, stop=True)
        nc.scalar.activation(biasEff[:], bp[:], AF.Identity, bias=k_fpb[:])
        FC2p = 2 * FC
        for d in range(NCH // 2):
            yp = psum.tile([20, FC2p], F32, tag="yp")
            for h in range(2):
                nc.tensor.matmul(yp[:, h * FC:(h + 1) * FC], fpw_eff[:],
                                 projT[:, d * FC2p + h * FC:d * FC2p + (h + 1) * FC],
                                 start=True, stop=True)
            nc.scalar.activation(yT[:, d * FC2p:(d + 1) * FC2p], yp[:],
                                 AF.Identity, bias=biasEff[:], scale=1.0,
                                 accum_out=scol3[:, d:d + 1])
            qscr2 = pool.tile([20, FC2p], BF16, tag="qscr2")
            nc.vector.scalar_tensor_tensor(
                qscr2[:], yT[:, d * FC2p:(d + 1) * FC2p], 1.0,
                yT[:, d * FC2p:(d + 1) * FC2p], ALU.mult, ALU.mult,
                accum_out=scol4[:, d:d + 1])
        nc.vector.reduce_sum(s2a[:, 0:1], scol3[:], axis=mybir.AxisListType.X)
        nc.vector.reduce_sum(s2a[:, 1:2], scol4[:], axis=mybir.AxisListType.X)
        nc.sync.dma_start(out=ar2_in[:, :], in_=s2a[:])

    with nc.semaphore("cc2") as cs:
        nc.gpsimd.collective_compute(
            "AllReduce", ALU.add, replica_groups=[list(range(NCORES))],
            ins=[ar2_in[:, :].opt()], outs=[ar2_out[:, :].opt()]).then_inc(cs, 1)
        nc.gpsimd.wait_ge(cs, 1)
        nc.all_engine_barrier()

    # ================= phase 3 =================
    with ExitStack() as ctx:
        tc = ctx.enter_context(SplitDrainTileContext(nc))
        pool = ctx.enter_context(tc.tile_pool(name="p3", bufs=3))
        psum = ctx.enter_context(tc.tile_pool(name="ps3", bufs=2, space="PSUM"))
        sq2 = pool.tile([20, 2], F32)
        nc.sync.dma_start(out=sq2[:], in_=ar2_out[:, :])
        mu2 = pool.tile([20, 1], F32)
        nc.scalar.mul(mu2[:], sq2[:, 0:1], 1.0 / B)
        mus2 = pool.tile([20, 1], F32)
        nc.vector.tensor_mul(mus2[:], mu2[:], mu2[:])
        var2 = pool.tile([20, 1], F32)
        nc.vector.scalar_tensor_tensor(var2[:], sq2[:, 1:2], 1.0 / B, mus2[:],
                                       ALU.mult, ALU.subtract)
        lnv2 = pool.tile([20, 1], F32)
        nc.scalar.activation(lnv2[:], var2[:], AF.Ln, bias=k_e2[:], scale=1.0)
        rst2 = pool.tile([20, 1], F32)
        nc.scalar.activation(rst2[:], lnv2[:], AF.Exp, bias=0.0, scale=-0.5)
        nc.vector.tensor_mul(a2v[:], rst2[:], k_g2[:])
        t2 = pool.tile([20, 1], F32)
        nc.vector.tensor_mul(t2[:], mu2[:], a2v[:])
        nc.vector.scalar_tensor_tensor(d2v[:], t2[:], -1.0, k_b2[:],
                                       ALU.mult, ALU.add)
        # 3a: z = gelu(a2*y+d2), z^2 — batched 4 chunks per op
        BL = 4 * FC
        for b in range(BS // BL):
            s = slice(b * BL, (b + 1) * BL)
            nc.scalar.activation(zzT[0:20, s], yT[:, s], AF.Gelu,
                                 bias=d2v[:], scale=a2v[:])
            nc.vector.tensor_mul(zzT[32:52, s], zzT[0:20, s], zzT[0:20, s])
        # 3b: memberships + head, pair-batched at FD=1024
        psum2 = ctx.enter_context(tc.tile_pool(name="ps3b", bufs=1, space="PSUM"))
        FC2 = 2 * FC
        for d in range(NCH // 2):
            zt = zzT[:, d * FC2:(d + 1) * FC2]
            u1 = psum.tile([128, FC2], F32, tag="u1")
            nc.tensor.matmul(u1[:, 0:FC], k_afz1[:], zt[:, 0:FC],
                             start=True, stop=True)
            nc.tensor.matmul(u1[:, FC:FC2], k_afz1[:], zt[:, FC:FC2],
                             start=True, stop=True)
            u2 = psum2.tile([72, FC2], F32, tag="u2")
            nc.tensor.matmul(u2[:, 0:FC], k_afz2[:], zt[:, 0:FC],
                             start=True, stop=True)
            nc.tensor.matmul(u2[:, FC:FC2], k_afz2[:], zt[:, FC:FC2],
                             start=True, stop=True)
            e1u = pool.tile([128, FC2], BF16, tag="e1u")
            nc.scalar.activation(e1u[:], u1[:], AF.Exp, bias=k_ub1[:], scale=-0.5)
            e1l = pool.tile([128, FC2], BF16, tag="e1l")
            nc.scalar.activation(e1l[:], u1[:], AF.Exp, bias=k_lb1[:],
                                 scale=k_ls1[:])
            e2u = pool.tile([72, FC2], BF16, tag="e2u")
            nc.scalar.activation(e2u[:], u2[:], AF.Exp, bias=k_ub2[:], scale=-0.5)
            t2l = pool.tile([72, FC2], F32, tag="t2l")
            nc.vector.tensor_scalar(t2l[:], u2[:], k_uc2[:], k_als2[:],
                                    ALU.min, ALU.mult)
            e2l = pool.tile([72, FC2], mybir.dt.int16, tag="e2l")
            nc.vector.tensor_scalar(e2l[:], t2l[:], k_ablb2[:], None, ALU.add)
            osb = pool.tile([1, FC2], F32, tag="osb")
            orow = psum2.tile([1, FC2], F32, tag="orow")
            for h in range(2):
                hs = slice(h * FC, (h + 1) * FC)
                ohs = orow[:, h * FC:(h + 1) * FC]
                nc.tensor.matmul(ohs, k_wh1[:], e1u[:, hs],
                                 start=True, stop=False)
                nc.tensor.matmul(ohs, k_wh1[:], e1l[:, hs],
                                 start=False, stop=False)
                nc.tensor.matmul(ohs, k_wh2[:], e2u[:, hs],
                                 start=False, stop=False)
                nc.tensor.matmul(ohs, k_wh2[:], e2l[:, hs].bitcast(BF16),
                                 start=False, stop=True)
            # batch row within pair = 8p + 4h + j  (orow col = h*512+j*128+p)
            nc.vector.tensor_scalar(
                osb[:].rearrange("one (p h j) -> one p h j", h=2, j=4),
                orow[:].rearrange("one (h j p) -> one p h j", h=2, j=4),
                head_b, None, ALU.add)
            ov = out[:, :].rearrange("(q s) one -> q (s one)", s=FC2)
            nc.sync.dma_start(out=ov[d:d + 1, :], in_=osb[:])
    octx.close()
    _split_multiwaits(nc)
    return nc


def _split_multiwaits(nc, max_waits=1):
    # hoist extra sync waits into single-wait nops placed just before the
    # offending instruction (walrus here rejects multi-wait instructions)
    for bb in nc.m.functions[0].blocks:
        insts = bb.instructions
        i = 0
        while i < len(insts):
            inst = insts[i]
            si = getattr(inst, "sync_info", None)
            waits = list(si.on_wait) if si and si.on_wait else []
            if len(waits) > max_waits:
                inst.sync_info = mybir.SyncInfo(
                    on_wait=waits[:max_waits], on_update=si.on_update)
                for j, w in enumerate(waits[max_waits:]):
                    n = mybir.InstNoOp(name=f"{inst.name}_ws{j}", ins=[], outs=[])
                    n.engine = inst.engine
                    n.sync_info = mybir.SyncInfo(on_wait=[w], on_update=[])
                    nc.register_instruction(n, overwrite=True)
                    insts.insert(i, n)
                    i += 1
            i += 1


LAST_RESULTS = None


def kernel(**inputs):
    global LAST_RESULTS
    import os
    x = np.asarray(inputs["x"], np.float32)
    p = {k: np.asarray(v) for k, v in inputs.items() if k != "x"}
    nc = _build(p)
    in_maps = [{"x": np.ascontiguousarray(x[i * BS:(i + 1) * BS])}
               for i in range(NCORES)]
    kw = {}
    if os.environ.get("KANFIS_TRACE") == "1":
        kw = dict(trace=True, tmpdir=os.environ.get("KANFIS_TRACE_DIR") or None)
    res = run_bass_kernel_spmd(nc, in_maps, core_ids=list(range(NCORES)), **kw)
    LAST_RESULTS = res
    return np.concatenate([res.results[i]["out"] for i in range(NCORES)], axis=0)


# revision 49
# speedup vs baseline: 1.0134x; 1.0003x over previous
"""KANFIS forward on 8 NeuronCores, data-parallel over the batch.

Per core (batch shard 16384 rows, processed as 16 chunk-pairs of 1024 so
every elementwise op runs at free-dim 1024 and amortizes engine overhead):

phase1  x (contiguous DMA; batch order permuted to col=j*128+p, undone at
        the output DMA) -> 8 PE transposes give x^T in PSUM partitions
        0:64; ACT Square writes the k=0 gaussian arg into 64:128; ONE
        packed ACT Exp produces [P ; rbf_0] (P = exp(dc*x/sigma^2), with
        per-partition scale switching function halves). The remaining 7
        RBF planes come from the squared-step DVE chain (bf16, 2x mode):
        X1=[t0;t1] via one mult + copies, PP2=[P^2;P^2], then
        X_{j+1} = X_j (.) PP2 yields plane pairs {t2,t3},{t4,t5},{t6,t7},
        where t_m = rbf_m / gamma_m and gamma folds into the baked bf16
        matmul weights. Group projection + k-sum = 4 paired bf16 matmuls
        + one float32r matmul for the linear term. BN1 partials: S1 via
        ACT accum on the PSUM->SBUF evict (applies proj bias, stores
        projT bf16), Q1 via batched ACT Square accum; per-pair partials
        land in scol columns, one DVE reduce at the end.
AR1     AllReduce [128,2]; a1 = g1*exp(-0.5*ln(var+eps)) (Ln+Exp share the
        natural_log_exp ACT table set - no table switch), d1 likewise.
phase2  BN1 affine folded into the fp weights: fpw_eff = a1 (.) fp_W^T
        (DVE), biasEff = fp_b + fp_W @ d1 (PE). Two bf16 matmuls per pair
        produce y^T [20,1024] feature-major; ACT evict applies biasEff and
        accumulates S2; DVE square-accum gives Q2.
AR2     AllReduce [20,2] -> a2,d2.
phase3  3a: z^T = Gelu(a2*y^T+d2) per-partition feature-major (no
        transposes), z^2 on DVE -> zzT [52,*] f32r (z at partitions 0:20,
        z^2 at 32:52 - DVE partition bases must be 32-aligned).
        3b: u = afz @ zz (float32r matmuls, 1 cyc/row); memberships: three
        ACT Exp -> bf16 tiles; the e2l plane instead uses a DVE int16
        bit-trick exp (bits16 = A16*clamped_exponent + B16 -> bitcast
        bf16, ~3% rel err, fine for the 2e-2 budget since the fuzzy head
        averages 400 such terms); head-sum via 8 bf16 matmuls into a
        two-bank [1,1024] PSUM row per pair, one DVE op adds head_b while
        un-permuting the batch order, one contiguous 4KB DMA per pair.

All parameters are baked into the NEFF as inline tensors; only x is a
runtime input. BN statistics are computed on device with two small
AllReduces. If the RBF grid is not uniformly spaced with per-group
constant sigma, a direct (non-recurrence) fallback path is built instead.

TimelineSim (cost-model) per-core estimate: ~191 us vs ~646 us for the
v0 baseline; hardware-verified rel err vs the fp64 reference: 3.8e-3.
"""
import numpy as np
import ml_dtypes
from contextlib import ExitStack

import concourse.bass as bass
import concourse.tile as tile
from concourse import mybir
from concourse.vector_clock import ScopedClock
from concourse.bass_utils import run_bass_kernel_spmd

F32 = mybir.dt.float32
F32R = mybir.dt.float32r
BF16 = mybir.dt.bfloat16
AF = mybir.ActivationFunctionType
ALU = mybir.AluOpType
BF = ml_dtypes.bfloat16

NCORES = 8
B = 131072
BS = B // NCORES          # 16384 rows per core
G, GS, K, O = 8, 8, 8, 16
TOT, R, FIN = 128, 10, 20
EPS = 1e-5
FC = 512                  # chunk free size
NCH = BS // FC            # 32 chunks


class SplitDrainTileContext(tile.TileContext):
    """walrus on this stack rejects >1 sync wait per instruction; split the
    kernel-tail drain's waits into single-wait nops."""

    def _drain_and_barrier(self, tick_clock, wait_clock):
        nc = self.nc
        nop = nc.sync.nop(nofuse=True)
        wait_clock.add_sem_waits(nop.ins, ScopedClock({None: tick_clock.global_clock}))
        si = nop.ins.sync_info
        waits = list(si.on_wait) if si and si.on_wait else []
        if len(waits) > 1:
            nop.ins.sync_info = mybir.SyncInfo(on_wait=waits[:1], on_update=si.on_update)
            for w in waits[1:]:
                n2 = nc.sync.nop(nofuse=True)
                n2.ins.sync_info = mybir.SyncInfo(on_wait=[w], on_update=[])
        nc.sync.drain()
        nc.all_engine_barrier()
        assert self.sems is not None
        popped = nc._tile_sem_poison_stack.pop()
        assert popped is self._sem_poison
        nc.clear_and_free_semaphores(list(self.sems.allocated().values()))
        nc.all_engine_barrier()


def _build(p):
    nc = bass.Bass(num_devices=NCORES)
    x = nc.dram_tensor("x", [BS, 64], F32, kind="ExternalInput")
    out = nc.dram_tensor("out", [BS, 1], F32, kind="ExternalOutput")
    ar1_in = nc.dram_tensor("ar1_in", [128, 2], F32)
    ar1_out = nc.dram_tensor("ar1_out", [128, 2], F32)
    ar2_in = nc.dram_tensor("ar2_in", [20, 2], F32)
    ar2_out = nc.dram_tensor("ar2_out", [20, 2], F32)

    # ---- baked constants (numpy) ----
    sig = np.exp(np.asarray(p["rbf_log_widths"], np.float64)) + 1e-6   # [G,K]
    cen = np.asarray(p["rbf_centres"], np.float64)                     # [G,K]
    inv = 1.0 / sig
    # recurrence structure: uniform centre spacing + k-constant sigma per group
    dcen = np.diff(cen, axis=1)
    rec_ok = (K >= 3 and np.allclose(dcen, dcen[:, :1], rtol=1e-5, atol=1e-7)
              and np.allclose(sig, sig[:, :1], rtol=1e-5, atol=1e-9))
    # seed Square scale/bias per partition.
    # Recurrence mode: one [128] vector — partitions 64:128 hold k=0 params
    # (Square input is x^T there; the squared result lands in 0:64).
    # The packed Exp then uses scale -0.5 on 0:64 (gaussian) and the P-scale
    # dc/sigma^2 on 64:128 (reads raw x^T).
    sqs = np.zeros((4, 128), np.float32)
    sqb = np.zeros((4, 128), np.float32)
    for j in range(K // 2):
        for half, k in ((0, 2 * j), (1, 2 * j + 1)):
            s = np.repeat(inv[:, k], GS)                # [64]
            b = np.repeat(-cen[:, k] * inv[:, k], GS)
            sqs[j, half * 64:(half + 1) * 64] = s
            sqb[j, half * 64:(half + 1) * 64] = b
    sqs_rec = np.zeros(128, np.float32)
    sqb_rec = np.zeros(128, np.float32)
    sqs_rec[:64] = np.repeat(inv[:, 0], GS)
    sqb_rec[:64] = np.repeat(-cen[:, 0] * inv[:, 0], GS)
    dc_g = dcen[:, 0] if rec_ok else np.zeros(G)
    exps_rec = np.zeros(128, np.float32)
    exps_rec[:64] = np.repeat(dc_g / sig[:, 0] ** 2, GS)
    exps_rec[64:] = -0.5
    # chain-plane weights: plane m is t_m = rbf_0 * P^m = rbf_m / gamma_m;
    # gamma_m = exp(-(c_m^2 - c_0^2) / (2 sigma^2)) folds into the lhsT.
    pw = np.asarray(p["proj_W"], np.float64)            # [G,O,GS]
    w = np.asarray(p["rbf_weights"], np.float64)        # [G,K]
    if rec_ok:
        gam = np.exp(-(cen ** 2 - cen[:, :1] ** 2) / (2 * sig[:, :1] ** 2))  # [G,K]
        weff = w * gam
    else:
        weff = w
    # lhsT blocks: 4 pairs {2j,2j+1} as [128,128]. In rec mode the planes are
    # t_m = rbf_m / gamma_m and gamma folds into weff.
    lhp = np.zeros((K // 2, 128, 128), np.float64)
    for j in range(K // 2):
        for half, k in ((0, 2 * j), (1, 2 * j + 1)):
            for g in range(G):
                lhp[j, half * 64 + g * GS:half * 64 + (g + 1) * GS,
                    g * O:(g + 1) * O] = pw[g].T * weff[g, k]
    linT = np.zeros((64, 128), np.float32)
    for g in range(G):
        linT[g * GS:(g + 1) * GS, g * O:(g + 1) * O] = (
            pw[g].T * np.asarray(p["rbf_linear_w"], np.float64)[g])
    pbv = np.asarray(p["proj_b"], np.float32).reshape(128, 1)
    # fuzzy layer
    su = np.exp(np.asarray(p["fz_log_su"], np.float64)) + 1e-6          # [R,FIN]
    sl = np.minimum(np.exp(np.asarray(p["fz_log_sl"], np.float64)) + 1e-6, su * 0.9)
    cz = np.asarray(p["fz_centres"], np.float64)
    # z rows at partitions 0:20, z^2 rows at 32:52 (DVE partition bases must
    # be 32-aligned, so z^2 is written at base 32; rows 20:32 stay zero)
    afz = np.zeros((52, 200), np.float32)
    for r in range(R):
        for f in range(FIN):
            m = r * FIN + f
            afz[f, m] = -2.0 * cz[r, f] / su[r, f] ** 2
            afz[32 + f, m] = 1.0 / su[r, f] ** 2
    ubias = (-0.5 * cz ** 2 / su ** 2).reshape(200, 1).astype(np.float32)
    lbias = (-0.5 * cz ** 2 / sl ** 2).reshape(200, 1).astype(np.float32)
    lscale = (-0.5 * (su / sl) ** 2).reshape(200, 1).astype(np.float32)
    wh = np.repeat(np.asarray(p["head_W"], np.float64).reshape(R, 1) * 0.5 / FIN,
                   FIN, 0).astype(np.float32)                           # [200,1]
    # bf16 bit-trick exp for the e2l plane: bits16 = A16*expnt + B16, expnt
    # clamped at -80 (below which the int16 pattern would go negative)
    A16 = 128.0 / np.log(2.0)
    B16 = 16256.0 - 486411.0 / 65536.0
    ls2 = lscale[128:, 0].astype(np.float64)
    lb2 = lbias[128:, 0].astype(np.float64)
    uc2 = ((-80.0 - lb2) / ls2).astype(np.float32).reshape(72, 1)
    als2 = (A16 * ls2).astype(np.float32).reshape(72, 1)
    ablb2 = (A16 * lb2 + B16).astype(np.float32).reshape(72, 1)
    head_b = float(np.asarray(p["head_b"]).reshape(-1)[0])

    def it(name, arr, dt=None):
        a = np.ascontiguousarray(arr)
        if dt == "bf16":
            a = a.astype(BF)
        else:
            a = a.astype(np.float32)
        return nc.inline_tensor(a, name=name)

    c_id = it("c_id", np.eye(128))
    c_sqs = it("c_sqs", sqs.T)           # [128,4]
    c_sqb = it("c_sqb", sqb.T)
    # startup-critical vectors in one DMA: cols = [sq scale, sq bias,
    # exp scale, proj bias]
    c_crit = it("c_crit", np.stack(
        [sqs_rec, sqb_rec, exps_rec, pbv[:, 0]], axis=1))
    c_lh = it("c_lh", lhp.transpose(1, 0, 2).reshape(128, 4 * 128), "bf16")
    c_lin = it("c_lin", linT)            # [64,128] f32
    c_pb = it("c_pb", pbv)
    c_g1 = it("c_g1", np.asarray(p["bn1_gamma"]).reshape(128, 1))
    c_b1 = it("c_b1", np.asarray(p["bn1_beta"]).reshape(128, 1))
    c_g2 = it("c_g2", np.asarray(p["bn2_gamma"]).reshape(20, 1))
    c_b2 = it("c_b2", np.asarray(p["bn2_beta"]).reshape(20, 1))
    c_fpw = it("c_fpw", np.asarray(p["fp_W"]).T)        # [128,20]
    c_fpb = it("c_fpb", np.asarray(p["fp_b"]).reshape(20, 1))
    c_afz1 = it("c_afz1", afz[:, :128])
    c_afz2 = it("c_afz2", afz[:, 128:])
    c_ub1 = it("c_ub1", ubias[:128]); c_ub2 = it("c_ub2", ubias[128:])
    c_lb1 = it("c_lb1", lbias[:128]); c_lb2 = it("c_lb2", lbias[128:])
    c_ls1 = it("c_ls1", lscale[:128]); c_ls2 = it("c_ls2", lscale[128:])
    c_wh1 = it("c_wh1", wh[:128], "bf16"); c_wh2 = it("c_wh2", wh[128:], "bf16")
    c_uc2 = it("c_uc2", uc2); c_als2 = it("c_als2", als2)
    c_ablb2 = it("c_ablb2", ablb2)

    octx = ExitStack()

    def sb(n, s, dt=F32):
        return octx.enter_context(nc.sbuf_tensor(n, s, dt))

    projT = sb("projT", [128, BS], BF16)     # 4MB persistent
    yT = sb("yT", [20, BS], BF16)
    zzT = sb("zzT", [52, BS], F32R)
    s1a = sb("s1a", [128, 2]); s2a = sb("s2a", [20, 2])
    scol1 = sb("scol1", [128, NCH // 2])     # per-pair S1 accums
    scol2 = sb("scol2", [128, NCH // 4])     # per-group Q1 accums
    scol3 = sb("scol3", [20, NCH // 2]); scol4 = sb("scol4", [20, NCH // 2])
    a1v = sb("a1v", [128, 1]); d1v = sb("d1v", [128, 1])
    a2v = sb("a2v", [20, 1]); d2v = sb("d2v", [20, 1])
    fpw_eff = sb("fpw_eff", [128, 20], BF16)
    biasEff = sb("biasEff", [20, 1])
    # const sbuf copies
    k_id = sb("k_id", [128, 128])
    k_sqs = sb("k_sqs", [128, 4]); k_sqb = sb("k_sqb", [128, 4])
    k_crit = sb("k_crit", [128, 4])
    k_lh = sb("k_lh", [128, 4 * 128], BF16)
    k_lin = sb("k_lin", [64, 128], F32R)
    k_g1 = sb("k_g1", [128, 1]); k_b1 = sb("k_b1", [128, 1])
    k_g2 = sb("k_g2", [20, 1]); k_b2 = sb("k_b2", [20, 1])
    k_fpw = sb("k_fpw", [128, 20]); k_fpb = sb("k_fpb", [20, 1])
    k_afz1 = sb("k_afz1", [52, 128], F32R); k_afz2 = sb("k_afz2", [52, 72], F32R)
    k_ub1 = sb("k_ub1", [128, 1]); k_ub2 = sb("k_ub2", [72, 1])
    k_lb1 = sb("k_lb1", [128, 1]); k_lb2 = sb("k_lb2", [72, 1])
    k_ls1 = sb("k_ls1", [128, 1]); k_ls2 = sb("k_ls2", [72, 1])
    k_wh1 = sb("k_wh1", [128, 1], BF16); k_wh2 = sb("k_wh2", [72, 1], BF16)
    k_uc2 = sb("k_uc2", [72, 1]); k_als2 = sb("k_als2", [72, 1])
    k_ablb2 = sb("k_ablb2", [72, 1])
    k_e1 = sb("k_e1", [128, 1]); k_e2 = sb("k_e2", [20, 1])
    k_hb4 = sb("k_hb4", [4, 1])

    # ================= phase 1 =================
    with ExitStack() as ctx:
        tc = ctx.enter_context(SplitDrainTileContext(nc))
        # consts via SWDGE in use order so they never block the x loads
        for dst, src in [(k_id, c_id), (k_crit, c_crit),
                         (k_lh, c_lh), (k_lin, c_lin),
                         (k_g1, c_g1),
                         (k_b1, c_b1), (k_g2, c_g2), (k_b2, c_b2),
                         (k_fpw, c_fpw), (k_fpb, c_fpb), (k_afz1, c_afz1),
                         (k_afz2, c_afz2), (k_ub1, c_ub1), (k_ub2, c_ub2),
                         (k_lb1, c_lb1), (k_lb2, c_lb2), (k_ls1, c_ls1),
                         (k_ls2, c_ls2), (k_wh1, c_wh1), (k_wh2, c_wh2),
                         (k_uc2, c_uc2), (k_als2, c_als2),
                         (k_ablb2, c_ablb2), (k_sqs, c_sqs), (k_sqb, c_sqb)]:
            cast = dst.dtype == F32R
            nc.gpsimd.dma_start(out=dst[:],
                                in_=src[:, :].bitcast(F32R) if cast else src[:, :])
        nc.vector.memset(k_e1[:], EPS)
        nc.vector.memset(k_e2[:], EPS)
        nc.vector.memset(k_hb4[:], head_b)
        pool = ctx.enter_context(tc.tile_pool(name="p1", bufs=3))
        psum = ctx.enter_context(tc.tile_pool(name="ps1", bufs=2, space="PSUM"))
        # PE clock warm-up: the HAM gate keeps the PE at half clock until
        # ~3.4us of sustained activity; burn that window on dummy matmuls
        # (garbage data, result never read, slot shared with the pp tag)
        warm = psum.tile([1, 2 * FC], F32, tag="pp", name="warm")
        for i in range(3):
            nc.tensor.matmul(warm[0:1, 0:FC], k_e1[0:52, 0:1],
                             zzT[:, i * FC:(i + 1) * FC].bitcast(F32),
                             start=True, stop=True)
        # chunk pairs: FD=1024 per op; within a pair, column j*128+p holds
        # batch row d*1024 + 8p + j (undone at the output DMA)
        FC2 = 2 * FC
        xv = x.rearrange("(d p s) f -> d p (s f)", p=128, s=8)
        for d in range(NCH // 2):
            xt = pool.tile([128, 512], F32, tag="xt")
            nc.sync.dma_start(out=xt[:], in_=xv[d])
            uix = psum.tile([128, FC2], F32, tag="uix")
            # x^T at partitions 0:64 (transpose may only write partition 0);
            # the squared gaussian arg goes to 64:128
            for j in range(8):
                nc.tensor.transpose(uix[0:64, j * 128:(j + 1) * 128],
                                    xt[:, j * 64:(j + 1) * 64], k_id[:])
            xts = pool.tile([64, FC2], F32R, tag="xts")
            if d % 2 == 0:
                nc.scalar.copy(xts[:], uix[0:64, :])
            else:
                nc.vector.tensor_copy(xts[:], uix[0:64, :])
            pp = psum.tile([128, FC2], F32, tag="pp")
            if rec_ok:
                nc.scalar.activation(uix[64:128, :], uix[0:64, :], AF.Square,
                                     bias=k_crit[0:64, 1:2],
                                     scale=k_crit[0:64, 0:1])
                # packed exp: [P ; rbf_0]
                e0 = pool.tile([128, FC2], BF16, tag="e0")
                nc.scalar.activation(e0[:], uix[:], AF.Exp, bias=0.0,
                                     scale=k_crit[:, 2:3])
                # P duplicated to partition base 64 (DVE needs equal input
                # bases when both tensors are in SBUF)
                ptU = pool.tile([128, FC2], BF16, tag="ptU")
                nc.vector.tensor_copy(ptU[64:128, :], e0[0:64, :])
                # squared-step chain: X1=[t0;t1], PP2=[P^2;P^2],
                # X_{j+1} = X_j * PP2 gives [t2;t3],[t4;t5],[t6;t7]
                # (gamma folded into lhsT)
                eA = pool.tile([128, FC2], BF16, tag="eA")
                eB = pool.tile([128, FC2], BF16, tag="eB")
                eC = pool.tile([128, FC2], BF16, tag="eC")
                eD = pool.tile([128, FC2], BF16, tag="eD")
                pp2 = pool.tile([128, FC2], BF16, tag="pp2")
                nc.vector.tensor_copy(eA[0:64, :], e0[64:128, :])
                nc.vector.tensor_tensor(eA[64:128, :], e0[64:128, :],
                                        ptU[64:128, :], ALU.mult)
                nc.vector.tensor_tensor(pp2[0:64, :], e0[0:64, :], e0[0:64, :],
                                        ALU.mult)
                nc.vector.tensor_copy(pp2[64:128, :], pp2[0:64, :])
                nc.vector.tensor_tensor(eB[:], eA[:], pp2[:], ALU.mult)
                nc.vector.tensor_tensor(eC[:], eB[:], pp2[:], ALU.mult)
                nc.vector.tensor_tensor(eD[:], eC[:], pp2[:], ALU.mult)
                for h in range(2):
                    hs = slice(h * FC, (h + 1) * FC)
                    for j, et in enumerate([eA, eB, eC, eD]):
                        nc.tensor.matmul(pp[:, hs],
                                         k_lh[:, j * 128:(j + 1) * 128],
                                         et[:, hs], start=(j == 0), stop=False)
                    nc.tensor.matmul(pp[:, hs], k_lin[:], xts[:, hs],
                                     start=False, stop=True)
            else:
                for j in range(8):
                    nc.tensor.matmul(uix[64:128, j * 128:(j + 1) * 128],
                                     xt[:, j * 64:(j + 1) * 64], k_id[:],
                                     start=True, stop=True)
                for h in range(2):
                    hs = slice(h * FC, (h + 1) * FC)
                    for j in range(K // 2):
                        uj2 = psum.tile([128, FC], F32, tag="uj", name="uj2")
                        nc.scalar.activation(uj2[:], uix[:, hs], AF.Square,
                                             bias=k_sqb[:, j:j + 1],
                                             scale=k_sqs[:, j:j + 1])
                        ej2 = pool.tile([128, FC], BF16, tag="ej2", name="ej2")
                        nc.scalar.activation(ej2[:], uj2[:], AF.Exp,
                                             bias=0.0, scale=-0.5)
                        nc.tensor.matmul(pp[:, hs],
                                         k_lh[:, j * 128:(j + 1) * 128],
                                         ej2[:], start=(j == 0), stop=False)
                    nc.tensor.matmul(pp[:, hs], k_lin[:], xts[:, hs],
                                     start=False, stop=True)
            nc.scalar.activation(projT[:, d * FC2:(d + 1) * FC2], pp[:],
                                 AF.Identity, bias=k_crit[:, 3:4], scale=1.0,
                                 accum_out=scol1[:, d:d + 1])
            if d % 2 == 1:
                qscr = pool.tile([128, 4 * FC], BF16, tag="qscr")
                g = d // 2
                nc.scalar.activation(qscr[:], projT[:, g * 4 * FC:(g + 1) * 4 * FC],
                                     AF.Square,
                                     accum_out=scol2[:, g:g + 1])
        nc.vector.reduce_sum(s1a[:, 0:1], scol1[:], axis=mybir.AxisListType.X)
        nc.vector.reduce_sum(s1a[:, 1:2], scol2[:], axis=mybir.AxisListType.X)
        nc.sync.dma_start(out=ar1_in[:, :], in_=s1a[:])

    with nc.semaphore("cc1") as cs:
        nc.gpsimd.collective_compute(
            "AllReduce", ALU.add, replica_groups=[list(range(NCORES))],
            ins=[ar1_in[:, :].opt()], outs=[ar1_out[:, :].opt()]).then_inc(cs, 1)
        nc.gpsimd.wait_ge(cs, 1)
        nc.all_engine_barrier()

    # ================= phase 2 =================
    with ExitStack() as ctx:
        tc = ctx.enter_context(SplitDrainTileContext(nc))
        pool = ctx.enter_context(tc.tile_pool(name="p2", bufs=3))
        psum = ctx.enter_context(tc.tile_pool(name="ps2", bufs=2, space="PSUM"))
        sq1 = pool.tile([128, 2], F32)
        nc.sync.dma_start(out=sq1[:], in_=ar1_out[:, :])
        mu = pool.tile([128, 1], F32)
        nc.scalar.mul(mu[:], sq1[:, 0:1], 1.0 / B)
        mus = pool.tile([128, 1], F32)
        nc.vector.tensor_mul(mus[:], mu[:], mu[:])
        var = pool.tile([128, 1], F32)
        nc.vector.scalar_tensor_tensor(var[:], sq1[:, 1:2], 1.0 / B, mus[:],
                                       ALU.mult, ALU.subtract)
        lnv = pool.tile([128, 1], F32)
        nc.scalar.activation(lnv[:], var[:], AF.Ln, bias=k_e1[:], scale=1.0)
        rst = pool.tile([128, 1], F32)
        nc.scalar.activation(rst[:], lnv[:], AF.Exp, bias=0.0, scale=-0.5)
        nc.vector.tensor_mul(a1v[:], rst[:], k_g1[:])
        t1 = pool.tile([128, 1], F32)
        nc.vector.tensor_mul(t1[:], mu[:], a1v[:])
        nc.vector.scalar_tensor_tensor(d1v[:], t1[:], -1.0, k_b1[:],
                                       ALU.mult, ALU.add)
        nc.vector.tensor_scalar(fpw_eff[:], k_fpw[:], a1v[:], None, ALU.mult)
        nc.gpsimd.memset(zzT[0:32, :].bitcast(F32), 0.0)
        bp = psum.tile([20, 1], F32, tag="bp")
        nc.tensor.matmul(bp[:], k_fpw[:], d1v[:], start=True, stop=True)
        nc.scalar.activation(biasEff[:], bp[:], AF.Identity, bias=k_fpb[:])
        FC2p = 2 * FC
        for d in range(NCH // 2):
            yp = psum.tile([20, FC2p], F32, tag="yp")
            for h in range(2):
                nc.tensor.matmul(yp[:, h * FC:(h + 1) * FC], fpw_eff[:],
                                 projT[:, d * FC2p + h * FC:d * FC2p + (h + 1) * FC],
                                 start=True, stop=True)
            nc.scalar.activation(yT[:, d * FC2p:(d + 1) * FC2p], yp[:],
                                 AF.Identity, bias=biasEff[:], scale=1.0,
                                 accum_out=scol3[:, d:d + 1])
            qscr2 = pool.tile([20, FC2p], BF16, tag="qscr2")
            nc.vector.scalar_tensor_tensor(
                qscr2[:], yT[:, d * FC2p:(d + 1) * FC2p], 1.0,
                yT[:, d * FC2p:(d + 1) * FC2p], ALU.mult, ALU.mult,
                accum_out=scol4[:, d:d + 1])
        nc.vector.reduce_sum(s2a[:, 0:1], scol3[:], axis=mybir.AxisListType.X)
        nc.vector.reduce_sum(s2a[:, 1:2], scol4[:], axis=mybir.AxisListType.X)
        nc.sync.dma_start(out=ar2_in[:, :], in_=s2a[:])

    with nc.semaphore("cc2") as cs:
        nc.gpsimd.collective_compute(
            "AllReduce", ALU.add, replica_groups=[list(range(NCORES))],
            ins=[ar2_in[:, :].opt()], outs=[ar2_out[:, :].opt()]).then_inc(cs, 1)
        nc.gpsimd.wait_ge(cs, 1)
        nc.all_engine_barrier()

    # ================= phase 3 =================
    with ExitStack() as ctx:
        tc = ctx.enter_context(SplitDrainTileContext(nc))
        pool = ctx.enter_context(tc.tile_pool(name="p3", bufs=3))
        psum = ctx.enter_context(tc.tile_pool(name="ps3", bufs=2, space="PSUM"))
        sq2 = pool.tile([20, 2], F32)
        nc.sync.dma_start(out=sq2[:], in_=ar2_out[:, :])
        mu2 = pool.tile([20, 1], F32)
        nc.scalar.mul(mu2[:], sq2[:, 0:1], 1.0 / B)
        mus2 = pool.tile([20, 1], F32)
        nc.vector.tensor_mul(mus2[:], mu2[:], mu2[:])
        var2 = pool.tile([20, 1], F32)
        nc.vector.scalar_tensor_tensor(var2[:], sq2[:, 1:2], 1.0 / B, mus2[:],
                                       ALU.mult, ALU.subtract)
        lnv2 = pool.tile([20, 1], F32)
        nc.scalar.activation(lnv2[:], var2[:], AF.Ln, bias=k_e2[:], scale=1.0)
        rst2 = pool.tile([20, 1], F32)
        nc.scalar.activation(rst2[:], lnv2[:], AF.Exp, bias=0.0, scale=-0.5)
        nc.vector.tensor_mul(a2v[:], rst2[:], k_g2[:])
        t2 = pool.tile([20, 1], F32)
        nc.vector.tensor_mul(t2[:], mu2[:], a2v[:])
        nc.vector.scalar_tensor_tensor(d2v[:], t2[:], -1.0, k_b2[:],
                                       ALU.mult, ALU.add)
        # 3a: z = gelu(a2*y+d2), z^2 — batched 4 chunks per op
        BL = 4 * FC
        for b in range(BS // BL):
            s = slice(b * BL, (b + 1) * BL)
            nc.scalar.activation(zzT[0:20, s], yT[:, s], AF.Gelu,
                                 bias=d2v[:], scale=a2v[:])
            nc.vector.tensor_mul(zzT[32:52, s], zzT[0:20, s], zzT[0:20, s])
        # 3b: memberships + head, pair-batched at FD=1024
        psum2 = ctx.enter_context(tc.tile_pool(name="ps3b", bufs=1, space="PSUM"))
        FC2 = 2 * FC
        for d in range(NCH // 2):
            zt = zzT[:, d * FC2:(d + 1) * FC2]
            u1 = psum.tile([128, FC2], F32, tag="u1")
            nc.tensor.matmul(u1[:, 0:FC], k_afz1[:], zt[:, 0:FC],
                             start=True, stop=True)
            nc.tensor.matmul(u1[:, FC:FC2], k_afz1[:], zt[:, FC:FC2],
                             start=True, stop=True)
            u2 = psum2.tile([72, FC2], F32, tag="u2")
            nc.tensor.matmul(u2[:, 0:FC], k_afz2[:], zt[:, 0:FC],
                             start=True, stop=True)
            nc.tensor.matmul(u2[:, FC:FC2], k_afz2[:], zt[:, FC:FC2],
                             start=True, stop=True)
            e1u = pool.tile([128, FC2], BF16, tag="e1u")
            nc.scalar.activation(e1u[:], u1[:], AF.Exp, bias=k_ub1[:], scale=-0.5)
            e1l = pool.tile([128, FC2], BF16, tag="e1l")
            nc.scalar.activation(e1l[:], u1[:], AF.Exp, bias=k_lb1[:],
                                 scale=k_ls1[:])
            e2u = pool.tile([72, FC2], BF16, tag="e2u")
            nc.scalar.activation(e2u[:], u2[:], AF.Exp, bias=k_ub2[:], scale=-0.5)
            t2l = pool.tile([72, FC2], F32, tag="t2l")
            nc.vector.tensor_scalar(t2l[:], u2[:], k_uc2[:], k_als2[:],
                                    ALU.min, ALU.mult)
            e2l = pool.tile([72, FC2], mybir.dt.int16, tag="e2l")
            nc.vector.tensor_scalar(e2l[:], t2l[:], k_ablb2[:], None, ALU.add)
            osb = pool.tile([1, FC2], F32, tag="osb")
            orow = psum2.tile([1, FC2], F32, tag="orow")
            for h in range(2):
                hs = slice(h * FC, (h + 1) * FC)
                ohs = orow[:, h * FC:(h + 1) * FC]
                nc.tensor.matmul(ohs, k_wh1[:], e1u[:, hs],
                                 start=True, stop=False)
                nc.tensor.matmul(ohs, k_wh1[:], e1l[:, hs],
                                 start=False, stop=False)
                nc.tensor.matmul(ohs, k_wh2[:], e2u[:, hs],
                                 start=False, stop=False)
                nc.tensor.matmul(ohs, k_wh2[:], e2l[:, hs].bitcast(BF16),
                                 start=False, stop=True)
            # batch row within pair = 8p + 4h + j  (orow col = h*512+j*128+p)
            nc.vector.tensor_scalar(
                osb[:].rearrange("one (p h j) -> one p h j", h=2, j=4),
                orow[:].rearrange("one (h j p) -> one p h j", h=2, j=4),
                head_b, None, ALU.add)
            ov = out[:, :].rearrange("(q s) one -> q (s one)", s=FC2)
            nc.sync.dma_start(out=ov[d:d + 1, :], in_=osb[:])
    octx.close()
    _split_multiwaits(nc)
    return nc


def _split_multiwaits(nc, max_waits=1):
    # hoist extra sync waits into single-wait nops placed just before the
    # offending instruction (walrus here rejects multi-wait instructions)
    for bb in nc.m.functions[0].blocks:
        insts = bb.instructions
        i = 0
        while i < len(insts):
            inst = insts[i]
            si = getattr(inst, "sync_info", None)
            waits = list(si.on_wait) if si and si.on_wait else []
            if len(waits) > max_waits:
                inst.sync_info = mybir.SyncInfo(
                    on_wait=waits[:max_waits], on_update=si.on_update)
                for j, w in enumerate(waits[max_waits:]):
                    n = mybir.InstNoOp(name=f"{inst.name}_ws{j}", ins=[], outs=[])
                    n.engine = inst.engine
                    n.sync_info = mybir.SyncInfo(on_wait=[w], on_update=[])
                    nc.register_instruction(n, overwrite=True)
                    insts.insert(i, n)
                    i += 1
            i += 1


LAST_RESULTS = None


def kernel(**inputs):
    global LAST_RESULTS
    import os
    x = np.asarray(inputs["x"], np.float32)
    p = {k: np.asarray(v) for k, v in inputs.items() if k != "x"}
    nc = _build(p)
    in_maps = [{"x": np.ascontiguousarray(x[i * BS:(i + 1) * BS])}
               for i in range(NCORES)]
    kw = {}
    if os.environ.get("KANFIS_TRACE") == "1":
        kw = dict(trace=True, tmpdir=os.environ.get("KANFIS_TRACE_DIR") or None)
    res = run_bass_kernel_spmd(nc, in_maps, core_ids=list(range(NCORES)), **kw)
    LAST_RESULTS = res
    return np.concatenate([res.results[i]["out"] for i in range(NCORES)], axis=0)
